# revision 33
# baseline (speedup 1.0000x reference)
"""GNN message-passing kernel for 8 Trainium2 NeuronCores (Bass/Tile).

Takes FULL inputs, shards nodes across 8 cores internally, runs the
4-layer GNN (dense -> spmm -> spmm -> dense) with two bf16 AllGathers
of the hidden node table, and PE-matmul-based weighted segment sums
(host-built one-hot selector matrices), then gathers the full output.
"""

import math
from contextlib import ExitStack
from dataclasses import dataclass

import ml_dtypes
import numpy as np

import concourse.bass as bass
import concourse.mybir as mybir
import concourse.tile as tile
from concourse import bacc
from concourse.bass_utils import run_bass_kernel_spmd
from concourse.masks import make_identity

BF16 = ml_dtypes.bfloat16
FP8 = ml_dtypes.float8_e4m3fn
AF = mybir.ActivationFunctionType


@dataclass(frozen=True)
class Cfg:
    n_nodes: int = 50000
    n_edges: int = 800000
    in_dim: int = 512
    h1: int = 512
    h2: int = 256
    out_dim: int = 128
    n_cores: int = 8
    group_blocks: int = 3  # row-blocks per gather group

    @property
    def nodes_per_core(self):
        return math.ceil(self.n_nodes / self.n_cores)

    @property
    def npad(self):  # per-core padded nodes
        return math.ceil(self.nodes_per_core / 128) * 128

    @property
    def nblocks(self):
        return self.npad // 128

    @property
    def ntot(self):
        return self.npad * self.n_cores

    @property
    def half(self):
        return self.ntot // 2

    @property
    def ngroups(self):
        return math.ceil(self.nblocks / self.group_blocks)


FULL = Cfg()


# ---------------------------------------------------------------- host prep


def edge_structure(cfg: Cfg, edge_row, edge_col, edge_weight):
    """Per-core edge streams with SPMD-uniform chunk counts.

    Returns (meta, per_core) where meta has the uniform chunk structure:
      meta['nch'][g][h]      total chunks in gather call (group g, half h)
      meta['chunk_blk'][g][h] list of block ids (one per chunk, ordered)
      meta['off16'][g][h]    idx-tile column offset (int16 cols) of the call
      meta['totch']          total chunks
      meta['idxcols']        total int16 columns of the idx tensor
    per_core[c] = dict(idx=[128, idxcols] int16, pmat=[128, totch*128] bf16)
    """
    nc_, npad, half, nb, G = (
        cfg.n_cores,
        cfg.npad,
        cfg.half,
        cfg.nblocks,
        cfg.group_blocks,
    )
    npc = cfg.nodes_per_core
    assert half <= 32767, "half-table must fit int16 indices"

    core_of = edge_row // npc
    lr_all = edge_row - core_of * npc  # local row
    # Block-aligned sub-table split: half 0 (A) holds every core's first
    # ceil(nb/2) row-blocks, half 1 (B) the rest, so table-half A can be
    # AllGathered as soon as each core finishes its first 25 blocks.
    # cl = position within the half-table.
    hh = ((nb + 1) // 2) * 128
    hB = npad - hh
    assert nc_ * hh <= 32767
    core_col = edge_col // npc
    loc = edge_col - core_col * npc
    half_all = (loc >= hh).astype(np.int64)
    cl_all = np.where(half_all == 0, core_col * hh + loc,
                      core_col * hB + (loc - hh))

    # bucket edges per (core, block, half)
    per = {}
    counts = np.zeros((nc_, nb, 2), np.int64)
    for c in range(nc_):
        m = core_of == c
        lr, cl, hf, w = lr_all[m], cl_all[m], half_all[m], edge_weight[m]
        blk = lr // 128
        order = np.lexsort((lr, hf, blk))
        per[c] = (lr[order], cl[order], hf[order], w[order], blk[order])
        np.add.at(counts[c], (blk, hf), 1)

    # uniform chunks per (block, half) = max over cores
    chunks_bh = np.ceil(counts / 128.0).astype(np.int64).max(axis=0)  # [nb, 2]

    ngroups = cfg.ngroups
    nch = [[0, 0] for _ in range(ngroups)]
    chunk_blk = [[[], []] for _ in range(ngroups)]
    off16 = [[0, 0] for _ in range(ngroups)]
    tot16 = 0
    totch = 0
    for g in range(ngroups):
        blocks = range(g * G, min((g + 1) * G, nb))
        for h in (0, 1):
            off16[g][h] = tot16
            n = 0
            for b in blocks:
                cb = int(chunks_bh[b, h])
                chunk_blk[g][h].extend([b] * cb)
                n += cb
            nch[g][h] = n
            tot16 += n * 8  # 128 idx per chunk -> 8 int16 cols
            totch += n

    meta = dict(
        nch=nch,
        chunk_blk=chunk_blk,
        off16=off16,
        totch=totch,
        idxcols=max(tot16, 8),
        chunks_bh=chunks_bh,
    )

    # SPMD-uniform chunk order within each (group, half): ascending
    # chunk-index-within-block (j), block-major within each j. Low-j chunks
    # are full for every core; high-j chunks carry the pad for every core,
    # so the pad concentrates at the call tail uniformly.
    # chunk_blk is rebuilt to match this order.
    for g in range(ngroups):
        blocks = list(range(g * G, min((g + 1) * G, nb)))
        for h in (0, 1):
            order = []
            maxcb = max((int(chunks_bh[b, h]) for b in blocks), default=0)
            for j in range(maxcb):
                for b in blocks:
                    if j < int(chunks_bh[b, h]):
                        order.append((b, j))
            meta["chunk_blk"][g][h] = [b for b, _ in order]
            meta.setdefault("chunk_ord", {})[(g, h)] = order

    # Pad slots: idx=-1 only in the TRAILING run of each <=15-chunk
    # sub-call (the gather ucode trims trailing negatives before
    # descriptor emission — saves Q7 time and DMA). Mid-call pad and the
    # first SAFE_GROUPS groups (whose SBUF tiles are uninitialized and
    # could hold NaN bytes; later groups reuse tiles holding valid old
    # values) use idx=0: gathers row 0, killed by zero pmat weight.
    SAFE_GROUPS = 999  # -1 trim disabled: trimmed calls hang the DMA sem protocol
    per_core = []
    for c in range(nc_):
        lr, cl, hf, w, blk = per[c]
        idx_flat = np.zeros(meta["idxcols"] * 16, np.int16)
        pmat = np.zeros((128, totch * 128), FP8)
        jchunk = 0
        for g in range(ngroups):
            blocks = list(range(g * G, min((g + 1) * G, nb)))
            for h in (0, 1):
                bdata = {}
                for b in blocks:
                    sel = (blk == b) & (hf == h)
                    bdata[b] = (cl[sel], lr[sel], w[sel])
                order = meta["chunk_ord"][(g, h)]
                ncall = len(order)
                base16 = meta["off16"][g][h]
                vals = np.zeros(ncall * 128, np.int64)
                real = np.zeros(ncall * 128, bool)
                for jj, (b, j) in enumerate(order):
                    e_cl, e_lr, e_w = bdata[b]
                    lo, hi = j * 128, min(j * 128 + 128, len(e_cl))
                    nreal = max(hi - lo, 0)
                    if nreal > 0:
                        vals[jj * 128 : jj * 128 + nreal] = e_cl[lo:hi]
                        real[jj * 128 : jj * 128 + nreal] = True
                        slot = np.arange(nreal)
                        r = e_lr[lo:hi] - b * 128
                        pmat[slot, (jchunk + jj) * 128 + r] = e_w[lo:hi].astype(
                            FP8
                        )
                jchunk += ncall
                if g >= SAFE_GROUPS:
                    # trailing trim per sub-call segment of 15 chunks
                    for seg in range(0, ncall, 26):
                        a, bnd = seg * 128, min(seg + 26, ncall) * 128
                        rseg = real[a:bnd]
                        nz = np.nonzero(rseg)[0]
                        last = nz[-1] + 1 if len(nz) else 0
                        # keep >=128 idx so every SDMA engine gets
                        # descriptors and the +16 completion sem fires
                        last = max(last, min(128, bnd - a))
                        vals[a + last : bnd] = -1
                i_in = np.arange(ncall * 128)
                idx_flat[(base16 + i_in // 16) * 16 + (i_in % 16)] = vals.astype(
                    np.int16
                )
        idx_mat = idx_flat.reshape(meta["idxcols"], 16).T  # [16, idxcols]
        idx_mat = np.tile(idx_mat, (8, 1))  # replicate to 128 partitions
        per_core.append(dict(idx=np.ascontiguousarray(idx_mat), pmat=pmat))

    return meta, per_core


def prep_inputs(cfg: Cfg, inputs):
    """Shard + lay out all per-core input tensors."""
    f = inputs["features"].astype(np.float32)
    meta, per_edge = edge_structure(
        cfg,
        inputs["edge_row"].astype(np.int64),
        inputs["edge_col"].astype(np.int64),
        inputs["edge_weight"].astype(np.float32),
    )
    kin = cfg.in_dim // 128
    k1 = cfg.h1 // 128
    k2 = cfg.h2 // 128

    def wlayout(w, kt):
        # [K, M] -> [128, kt*M] with [:, i*M:(i+1)*M] = w[i*128:(i+1)*128]
        K, M = w.shape
        return (
            w.reshape(kt, 128, M).transpose(1, 0, 2).reshape(128, kt * M)
        ).astype(BF16)

    w1 = wlayout(inputs["W_lin1"].astype(np.float32), kin)
    wg1 = wlayout(inputs["W_g1"].astype(np.float32), k1)
    wg2 = wlayout(inputs["W_g2"].astype(np.float32), k2)
    wl2 = wlayout(inputs["W_lin2"].astype(np.float32), k2)
    b1 = (
        inputs["b_lin1"].astype(np.float32).reshape(kin, 128).T.copy()
    )  # [128, kin]
    bg1 = inputs["b_g1"].astype(BF16).reshape(1, cfg.h2)
    bg2 = inputs["b_g2"].astype(BF16).reshape(1, cfg.h2)
    bl2 = inputs["b_lin2"].astype(BF16).reshape(1, cfg.out_dim)

    npc, npad = cfg.nodes_per_core, cfg.npad
    in_maps = []
    for c in range(cfg.n_cores):
        lo = c * npc
        hi = min((c + 1) * npc, cfg.n_nodes)
        xc = np.zeros((npad, cfg.in_dim), np.float32)
        xc[: hi - lo] = f[lo:hi]
        # XT layout [128, kin*npad]: [:, i*npad:(i+1)*npad] = x.T[i*128:...]
        xt = (
            xc.T.reshape(kin, 128, npad)
            .transpose(1, 0, 2)
            .reshape(128, kin * npad)
        ).astype(BF16)
        in_maps.append(
            {
                "xt": np.ascontiguousarray(xt),
                "w1": w1,
                "wg1": wg1,
                "wg2": wg2,
                "wl2": wl2,
                "b1": b1,
                "bg1": bg1,
                "bg2": bg2,
                "bl2": bl2,
                "idx": per_edge[c]["idx"],
                "pmat": per_edge[c]["pmat"],
            }
        )
    return meta, in_maps


# ---------------------------------------------------------------- kernel IR


def build(cfg: Cfg, meta):
    nc = bacc.Bacc(
        "TRN2",
        target_bir_lowering=False,
        debug=False,
        num_devices=cfg.n_cores,
        num_swdge_queues=4,
    )
    bf = mybir.dt.bfloat16
    f8 = mybir.dt.float8e4
    f32 = mybir.dt.float32
    i16 = mybir.dt.int16
    kin = cfg.in_dim // 128
    k1 = cfg.h1 // 128
    k2 = cfg.h2 // 128
    npad, nb, G, H2, OUT = (
        cfg.npad,
        cfg.nblocks,
        cfg.group_blocks,
        cfg.h2,
        cfg.out_dim,
    )
    HALF = cfg.half
    totch = meta["totch"]

    xt_d = nc.dram_tensor("xt", [128, kin * npad], bf, kind="ExternalInput").ap()
    w1_d = nc.dram_tensor("w1", [128, kin * cfg.h1], bf, kind="ExternalInput").ap()
    wg1_d = nc.dram_tensor("wg1", [128, k1 * H2], bf, kind="ExternalInput").ap()
    wg2_d = nc.dram_tensor("wg2", [128, k2 * H2], bf, kind="ExternalInput").ap()
    wl2_d = nc.dram_tensor("wl2", [128, k2 * OUT], bf, kind="ExternalInput").ap()
    b1_d = nc.dram_tensor("b1", [128, kin], f32, kind="ExternalInput").ap()
    bg1_d = nc.dram_tensor("bg1", [1, H2], bf, kind="ExternalInput").ap()
    bg2_d = nc.dram_tensor("bg2", [1, H2], bf, kind="ExternalInput").ap()
    bl2_d = nc.dram_tensor("bl2", [1, OUT], bf, kind="ExternalInput").ap()
    idx_d = nc.dram_tensor(
        "idx", [128, meta["idxcols"]], i16, kind="ExternalInput"
    ).ap()
    pmat_d = nc.dram_tensor(
        "pmat", [128, totch * 128], f8, kind="ExternalInput"
    ).ap()
    y_d = nc.dram_tensor("y", [npad, OUT], f32, kind="ExternalOutput").ap()

    hh = ((nb + 1) // 2) * 128  # A-half rows per core (block aligned)
    hB = npad - hh
    nbA = hh // 128  # blocks in A half
    HA = cfg.n_cores * hh
    HB = cfg.n_cores * hB

    g1_localA = nc.dram_tensor("g1_localA", [hh, H2], f8).ap()
    g1_localB = nc.dram_tensor("g1_localB", [hB, H2], f8).ap()
    g2_localA = nc.dram_tensor("g2_localA", [hh, H2], f8).ap()
    g2_localB = nc.dram_tensor("g2_localB", [hB, H2], f8).ap()
    g1_tableA = nc.dram_tensor("g1_tableA", [HA, H2], f8, addr_space="Shared").ap()
    g1_tableB = nc.dram_tensor("g1_tableB", [HB, H2], f8, addr_space="Shared").ap()
    g2_tableA = nc.dram_tensor("g2_tableA", [HA, H2], f8, addr_space="Shared").ap()
    g2_tableB = nc.dram_tensor("g2_tableB", [HB, H2], f8, addr_space="Shared").ap()

    rg = [list(range(cfg.n_cores))]

    def spmm(tc, ctx, nc, tables, idx_s, ones_t, brow, psum_tag, out_cb,
             after_group=None):
        """Weighted segment-sum of gathered table rows, per row-block.

        h0 gathers are issued S groups ahead of h1 so the first h1 gather
        (which waits for the B-half AllGather) doesn't starve the pipeline.
        """
        S = 5
        gp = [
            ctx.enter_context(
                tc.tile_pool(name=f"gath{psum_tag}{h}", bufs=(S + 2 if h == 0 else 4))
            )
            for h in (0, 1)
        ]
        pp = ctx.enter_context(tc.tile_pool(name=f"pm{psum_tag}", bufs=4))
        sp = ctx.enter_context(
            tc.tile_pool(name=f"ps{psum_tag}", bufs=2 * G, space="PSUM")
        )
        qstate = [0]

        def issue(g, h):
            n = meta["nch"][g][h]
            if n == 0:
                return None
            t = gp[h].tile([128, n, H2], f8, tag=f"g{h}")
            # split into <=15-chunk (1920-idx) sub-calls: a single
            # dma_gather must fit the SWDGE descriptor ring. Rotate
            # across the 4 SWDGE queues so descriptor generation runs
            # on all 4 Q7 core pairs concurrently.
            for lo in range(0, n, 26):
                ns = min(26, n - lo)
                o16 = meta["off16"][g][h] + lo * 8
                nc.gpsimd.dma_gather(
                    out_ap=t[:, lo : lo + ns, :],
                    in_ap=tables[h][:, :],
                    idxs_ap=idx_s[:, o16 : o16 + ns * 8],
                    num_idxs=ns * 128,
                    num_idxs_reg=ns * 128,
                    elem_size=H2,
                    single_packet=False,
                    queue_num=qstate[0] % 4,
                )
                qstate[0] += 1
            return t

        pend0 = {}
        for g in range(min(S, cfg.ngroups)):
            pend0[g] = issue(g, 0)
        j0 = 0
        for g in range(cfg.ngroups):
            blocks = list(range(g * G, min((g + 1) * G, nb)))
            gt = {0: pend0.pop(g)}
            if g + S < cfg.ngroups:
                pend0[g + S] = issue(g + S, 0)
            gt[1] = issue(g, 1)
            chg = meta["nch"][g][0] + meta["nch"][g][1]
            if chg > 0:
                ptile = pp.tile([128, chg * 128], f8, tag="p")
                nc.sync.dma_start(
                    ptile[:], pmat_d[:, j0 * 128 : (j0 + chg) * 128]
                )
            psums = {
                b: sp.tile([128, H2], f32, tag="ps", name=f"ps{psum_tag}_{b}")
                for b in blocks
            }
            started = dict.fromkeys(blocks, False)
            jj = 0
            for h in (0, 1):
                for jh, b in enumerate(meta["chunk_blk"][g][h]):
                    nc.tensor.matmul(
                        psums[b][:],
                        lhsT=ptile[:, jj * 128 : (jj + 1) * 128],
                        rhs=gt[h][:, jh, :],
                        start=not started[b],
                        stop=False,
                    )
                    started[b] = True
                    jj += 1
            for b in blocks:
                nc.tensor.matmul(
                    psums[b][:],
                    lhsT=ones_t[:1, :],
                    rhs=brow[:1, :],
                    start=not started[b],
                    stop=True,
                )
                out_cb(b, psums[b])
            if after_group is not None:
                after_group(g, blocks)
            j0 += chg

    with tile.TileContext(nc) as tc:
        with ExitStack() as top:
            const = top.enter_context(tc.tile_pool(name="const", bufs=1))
            w1_s = const.tile([128, kin * cfg.h1], bf)
            nc.sync.dma_start(w1_s[:], w1_d[:, :])
            wg1_s = const.tile([128, k1 * H2], bf)
            nc.sync.dma_start(wg1_s[:], wg1_d[:, :])
            wg2_s = const.tile([128, k2 * H2], bf)
            nc.sync.dma_start(wg2_s[:], wg2_d[:, :])
            wl2_s = const.tile([128, k2 * OUT], bf)
            nc.sync.dma_start(wl2_s[:], wl2_d[:, :])
            b1_s = const.tile([128, kin], f32)
            nc.sync.dma_start(b1_s[:], b1_d[:, :])
            bg1_s = const.tile([1, H2], bf)
            nc.sync.dma_start(bg1_s[:], bg1_d[:, :])
            bg2_s = const.tile([1, H2], bf)
            nc.sync.dma_start(bg2_s[:], bg2_d[:, :])
            bl2_s = const.tile([1, OUT], bf)
            nc.sync.dma_start(bl2_s[:], bl2_d[:, :])
            idx_s = const.tile([128, meta["idxcols"]], i16)
            nc.sync.dma_start(idx_s[:], idx_d[:, :])
            ident = const.tile([128, 128], bf)
            make_identity(nc, ident[:])
            ones_t = const.tile([1, 128], bf)
            nc.gpsimd.memset(ones_t[:], 1.0)

            # ---------------- L1: h1T[f, n] = sigmoid(W1.T @ X.T + b1)
            with ExitStack() as ph1:
                h1p = ph1.enter_context(tc.tile_pool(name="h1t", bufs=1))
                h1t = h1p.tile([128, k1 * npad], bf)
                with ExitStack() as px:
                    xp = px.enter_context(tc.tile_pool(name="xt", bufs=1))
                    psp = px.enter_context(
                        tc.tile_pool(name="ps1", bufs=4, space="PSUM")
                    )
                    xt_k = []
                    for kt in range(kin):
                        xk = xp.tile([128, npad], bf, name=f"xt{kt}")
                        nc.sync.dma_start(
                            xk[:], xt_d[:, kt * npad : (kt + 1) * npad]
                        )
                        xt_k.append(xk)
                    nsl = [(i * 512, min((i + 1) * 512, npad)) for i in range(math.ceil(npad / 512))]
                    for f1t in range(k1):
                        for a, b_ in nsl:
                            nw = b_ - a
                            ps = psp.tile([128, 512], f32, tag="ps")
                            for kt in range(kin):
                                nc.tensor.matmul(
                                    ps[:, :nw],
                                    lhsT=w1_s[
                                        :,
                                        kt * cfg.h1
                                        + f1t * 128 : kt * cfg.h1
                                        + f1t * 128
                                        + 128,
                                    ],
                                    rhs=xt_k[kt][:, a:b_],
                                    start=(kt == 0),
                                    stop=(kt == kin - 1),
                                )
                            nc.scalar.activation(
                                h1t[:, f1t * npad + a : f1t * npad + b_],
                                ps[:, :nw],
                                AF.Sigmoid,
                                bias=b1_s[:, f1t : f1t + 1],
                            )

                # ---------------- L2a: g1[n, h2] = h1 @ Wg1  (lhsT = h1T)
                def store_half(local_a, local_b, b, tile_):
                    if b < nbA:
                        nc.sync.dma_start(
                            local_a[b * 128 : (b + 1) * 128, :], tile_[:]
                        )
                    else:
                        bb = b - nbA
                        nc.sync.dma_start(
                            local_b[bb * 128 : (bb + 1) * 128, :], tile_[:]
                        )

                def allgather(ins_, outs_):
                    nc.gpsimd.collective_compute(
                        "AllGather",
                        mybir.AluOpType.bypass,
                        replica_groups=rg,
                        ins=[ins_],
                        outs=[outs_],
                    )

                with ExitStack() as p2:
                    psp2 = p2.enter_context(
                        tc.tile_pool(name="ps2", bufs=4, space="PSUM")
                    )
                    tp2 = p2.enter_context(tc.tile_pool(name="g1t", bufs=3))
                    for b in range(nb):
                        ps = psp2.tile([128, H2], f32, tag="ps")
                        for kt in range(k1):
                            nc.tensor.matmul(
                                ps[:],
                                lhsT=h1t[
                                    :, kt * npad + b * 128 : kt * npad + b * 128 + 128
                                ],
                                rhs=wg1_s[:, kt * H2 : (kt + 1) * H2],
                                start=(kt == 0),
                                stop=(kt == k1 - 1),
                            )
                        g1tile = tp2.tile([128, H2], f8, tag="g1")
                        nc.vector.tensor_copy(g1tile[:], ps[:])
                        store_half(g1_localA, g1_localB, b, g1tile)
                        if b == nbA - 1:
                            allgather(g1_localA[:, :], g1_tableA[:, :])
                    allgather(g1_localB[:, :], g1_tableB[:, :])

            # ---------------- spmm1 -> h2, L3a (g2) fused per block,
            # AG2 halves issued as soon as their blocks are stored
            with ExitStack() as ph2:
                h2p = ph2.enter_context(tc.tile_pool(name="h2res", bufs=1))
                h2r = h2p.tile([128, nb * H2], bf)
                tps = ph2.enter_context(
                    tc.tile_pool(name="tps", bufs=1, space="PSUM")
                )
                psp3 = ph2.enter_context(
                    tc.tile_pool(name="ps3", bufs=1, space="PSUM")
                )
                tp3 = ph2.enter_context(tc.tile_pool(name="l3t", bufs=4))

                with ExitStack() as ps1:
                    def cb1(b, psum):
                        nc.scalar.activation(
                            h2r[:, b * H2 : (b + 1) * H2], psum[:], AF.Relu
                        )
                        h2T = tp3.tile([128, k2, 128], bf, tag="h2T")
                        for kt in range(k2):
                            pt = tps.tile([128, 128], bf, tag="pt")
                            nc.tensor.transpose(
                                pt[:],
                                h2r[:, b * H2 + kt * 128 : b * H2 + (kt + 1) * 128],
                                ident[:],
                            )
                            nc.vector.tensor_copy(h2T[:, kt, :], pt[:])
                        ps = psp3.tile([128, H2], f32, tag="ps")
                        for kt in range(k2):
                            nc.tensor.matmul(
                                ps[:],
                                lhsT=h2T[:, kt, :],
                                rhs=wg2_s[:, kt * H2 : (kt + 1) * H2],
                                start=(kt == 0),
                                stop=(kt == k2 - 1),
                            )
                        g2tile = tp3.tile([128, H2], f8, tag="g2")
                        nc.vector.tensor_copy(g2tile[:], ps[:])
                        store_half(g2_localA, g2_localB, b, g2tile)

                    spmm(tc, ps1, nc, (g1_tableA, g1_tableB), idx_s, ones_t,
                         bg1_s, "a", cb1)

                # both AG2 halves after the full spmm1/L3a stream: a
                # mid-stream collective trigger blocks the gpsimd queue on
                # cross-core rendezvous and stalls the gather pipeline
                allgather(g2_localA[:, :], g2_tableA[:, :])
                allgather(g2_localB[:, :], g2_tableB[:, :])

            # ---------------- spmm2 + L4 fused per block
            with ExitStack() as ps2x:
                tps4 = ps2x.enter_context(
                    tc.tile_pool(name="tps4", bufs=1, space="PSUM")
                )
                psp4 = ps2x.enter_context(
                    tc.tile_pool(name="ps4", bufs=1, space="PSUM")
                )
                tp4 = ps2x.enter_context(tc.tile_pool(name="l4t", bufs=3))

                def cb2(b, psum):
                    h3t = tp4.tile([128, H2], bf, tag="h3")
                    nc.scalar.activation(h3t[:], psum[:], AF.Relu)
                    h3T = tp4.tile([128, k2, 128], bf, tag="h3T")
                    for kt in range(k2):
                        pt = tps4.tile([128, 128], bf, tag="pt")
                        nc.tensor.transpose(
                            pt[:], h3t[:, kt * 128 : (kt + 1) * 128], ident[:]
                        )
                        nc.vector.tensor_copy(h3T[:, kt, :], pt[:])
                    ps4 = psp4.tile([128, OUT], f32, tag="ps")
                    for kt in range(k2):
                        nc.tensor.matmul(
                            ps4[:],
                            lhsT=h3T[:, kt, :],
                            rhs=wl2_s[:, kt * OUT : (kt + 1) * OUT],
                            start=(kt == 0),
                            stop=False,
                        )
                    nc.tensor.matmul(
                        ps4[:],
                        lhsT=ones_t[:1, :],
                        rhs=bl2_s[:1, :],
                        start=False,
                        stop=True,
                    )
                    yt = tp4.tile([128, OUT], f32, tag="y")
                    nc.vector.tensor_copy(yt[:], ps4[:])
                    nc.sync.dma_start(y_d[b * 128 : (b + 1) * 128, :], yt[:])

                spmm(tc, ps2x, nc, (g2_tableA, g2_tableB), idx_s, ones_t,
                     bg2_s, "b", cb2)

    nc.compile()
    return nc


# ---------------------------------------------------------------- driver

_CACHE = {}


def run(inputs, cfg: Cfg = FULL, trace=False, tmpdir=None):
    meta, in_maps = prep_inputs(cfg, inputs)
    key = (cfg, meta["totch"], meta["idxcols"])
    if key not in _CACHE:
        _CACHE[key] = build(cfg, meta)
    nc = _CACHE[key]
    res = run_bass_kernel_spmd(
        nc,
        in_maps,
        core_ids=list(range(cfg.n_cores)),
        trace=trace,
        tmpdir=tmpdir,
    )
    npc = cfg.nodes_per_core
    out = np.empty((cfg.n_nodes, cfg.out_dim), np.float32)
    for c in range(cfg.n_cores):
        lo = c * npc
        hi = min((c + 1) * npc, cfg.n_nodes)
        out[lo:hi] = res.results[c]["y"][: hi - lo]
    return out, res


def kernel(**inputs) -> np.ndarray:
    out, _ = run(inputs, FULL, trace=False)
    return out



# revision 34
# speedup vs baseline: 1.0844x; 1.0844x over previous
"""GNN message-passing kernel for 8 Trainium2 NeuronCores (Bass/Tile).

Takes FULL inputs, shards nodes across 8 cores internally, runs the
4-layer GNN (dense -> spmm -> spmm -> dense) with two bf16 AllGathers
of the hidden node table, and PE-matmul-based weighted segment sums
(host-built one-hot selector matrices), then gathers the full output.
"""

import math
from contextlib import ExitStack
from dataclasses import dataclass

import ml_dtypes
import numpy as np

import concourse.bass as bass
import concourse.mybir as mybir
import concourse.tile as tile
from concourse import bacc
from concourse.bass_utils import run_bass_kernel_spmd
from concourse.masks import make_identity

BF16 = ml_dtypes.bfloat16
FP8 = ml_dtypes.float8_e4m3fn
AF = mybir.ActivationFunctionType


@dataclass(frozen=True)
class Cfg:
    n_nodes: int = 50000
    n_edges: int = 800000
    in_dim: int = 512
    h1: int = 512
    h2: int = 256
    out_dim: int = 128
    n_cores: int = 8
    group_blocks: int = 3  # row-blocks per gather group

    @property
    def nodes_per_core(self):
        return math.ceil(self.n_nodes / self.n_cores)

    @property
    def npad(self):  # per-core padded nodes
        return math.ceil(self.nodes_per_core / 128) * 128

    @property
    def nblocks(self):
        return self.npad // 128

    @property
    def ntot(self):
        return self.npad * self.n_cores

    @property
    def half(self):
        return self.ntot // 2

    @property
    def ngroups(self):
        return math.ceil(self.nblocks / self.group_blocks)


FULL = Cfg()


# ---------------------------------------------------------------- host prep


def edge_structure(cfg: Cfg, edge_row, edge_col, edge_weight):
    """Per-core edge streams with SPMD-uniform chunk counts.

    Returns (meta, per_core) where meta has the uniform chunk structure:
      meta['nch'][g][h]      total chunks in gather call (group g, half h)
      meta['chunk_blk'][g][h] list of block ids (one per chunk, ordered)
      meta['off16'][g][h]    idx-tile column offset (int16 cols) of the call
      meta['totch']          total chunks
      meta['idxcols']        total int16 columns of the idx tensor
    per_core[c] = dict(idx=[128, idxcols] int16, pmat=[128, totch*128] bf16)
    """
    nc_, npad, half, nb, G = (
        cfg.n_cores,
        cfg.npad,
        cfg.half,
        cfg.nblocks,
        cfg.group_blocks,
    )
    npc = cfg.nodes_per_core
    assert half <= 32767, "half-table must fit int16 indices"

    core_of = edge_row // npc
    lr_all = edge_row - core_of * npc  # local row
    # Block-aligned sub-table split: half 0 (A) holds every core's first
    # ceil(nb/2) row-blocks, half 1 (B) the rest, so table-half A can be
    # AllGathered as soon as each core finishes its first 25 blocks.
    # cl = position within the half-table.
    hh = ((nb + 1) // 2) * 128
    hB = npad - hh
    assert nc_ * hh <= 32767
    core_col = edge_col // npc
    loc = edge_col - core_col * npc
    half_all = (loc >= hh).astype(np.int64)
    cl_all = np.where(half_all == 0, core_col * hh + loc,
                      core_col * hB + (loc - hh))

    # bucket edges per (core, block, half)
    per = {}
    counts = np.zeros((nc_, nb, 2), np.int64)
    for c in range(nc_):
        m = core_of == c
        lr, cl, hf, w = lr_all[m], cl_all[m], half_all[m], edge_weight[m]
        blk = lr // 128
        order = np.lexsort((lr, hf, blk))
        per[c] = (lr[order], cl[order], hf[order], w[order], blk[order])
        np.add.at(counts[c], (blk, hf), 1)

    # uniform chunks per (block, half) = max over cores
    chunks_bh = np.ceil(counts / 128.0).astype(np.int64).max(axis=0)  # [nb, 2]

    ngroups = cfg.ngroups
    nch = [[0, 0] for _ in range(ngroups)]
    chunk_blk = [[[], []] for _ in range(ngroups)]
    off16 = [[0, 0] for _ in range(ngroups)]
    tot16 = 0
    totch = 0
    for g in range(ngroups):
        blocks = range(g * G, min((g + 1) * G, nb))
        for h in (0, 1):
            off16[g][h] = tot16
            n = 0
            for b in blocks:
                cb = int(chunks_bh[b, h])
                chunk_blk[g][h].extend([b] * cb)
                n += cb
            nch[g][h] = n
            tot16 += n * 8  # 128 idx per chunk -> 8 int16 cols
            totch += n

    meta = dict(
        nch=nch,
        chunk_blk=chunk_blk,
        off16=off16,
        totch=totch,
        idxcols=max(tot16, 8),
        chunks_bh=chunks_bh,
    )

    # SPMD-uniform chunk order within each (group, half): ascending
    # chunk-index-within-block (j), block-major within each j. Low-j chunks
    # are full for every core; high-j chunks carry the pad for every core,
    # so the pad concentrates at the call tail uniformly.
    # chunk_blk is rebuilt to match this order.
    for g in range(ngroups):
        blocks = list(range(g * G, min((g + 1) * G, nb)))
        for h in (0, 1):
            order = []
            maxcb = max((int(chunks_bh[b, h]) for b in blocks), default=0)
            for j in range(maxcb):
                for b in blocks:
                    if j < int(chunks_bh[b, h]):
                        order.append((b, j))
            meta["chunk_blk"][g][h] = [b for b, _ in order]
            meta.setdefault("chunk_ord", {})[(g, h)] = order

    # Pad slots: idx=-1 only in the TRAILING run of each <=15-chunk
    # sub-call (the gather ucode trims trailing negatives before
    # descriptor emission — saves Q7 time and DMA). Mid-call pad and the
    # first SAFE_GROUPS groups (whose SBUF tiles are uninitialized and
    # could hold NaN bytes; later groups reuse tiles holding valid old
    # values) use idx=0: gathers row 0, killed by zero pmat weight.
    SAFE_GROUPS = 999  # -1 trim disabled: trimmed calls hang the DMA sem protocol
    per_core = []
    for c in range(nc_):
        lr, cl, hf, w, blk = per[c]
        idx_flat = np.zeros(meta["idxcols"] * 16, np.int16)
        pmat = np.zeros((128, totch * 128), FP8)
        jchunk = 0
        for g in range(ngroups):
            blocks = list(range(g * G, min((g + 1) * G, nb)))
            for h in (0, 1):
                bdata = {}
                for b in blocks:
                    sel = (blk == b) & (hf == h)
                    bdata[b] = (cl[sel], lr[sel], w[sel])
                order = meta["chunk_ord"][(g, h)]
                ncall = len(order)
                base16 = meta["off16"][g][h]
                vals = np.zeros(ncall * 128, np.int64)
                real = np.zeros(ncall * 128, bool)
                for jj, (b, j) in enumerate(order):
                    e_cl, e_lr, e_w = bdata[b]
                    lo, hi = j * 128, min(j * 128 + 128, len(e_cl))
                    nreal = max(hi - lo, 0)
                    if nreal > 0:
                        vals[jj * 128 : jj * 128 + nreal] = e_cl[lo:hi]
                        real[jj * 128 : jj * 128 + nreal] = True
                        slot = np.arange(nreal)
                        r = e_lr[lo:hi] - b * 128
                        pmat[slot, (jchunk + jj) * 128 + r] = e_w[lo:hi].astype(
                            FP8
                        )
                jchunk += ncall
                if g >= SAFE_GROUPS:
                    # trailing trim per sub-call segment of 15 chunks
                    for seg in range(0, ncall, 15):
                        a, bnd = seg * 128, min(seg + 15, ncall) * 128
                        rseg = real[a:bnd]
                        nz = np.nonzero(rseg)[0]
                        last = nz[-1] + 1 if len(nz) else 0
                        # keep >=128 idx so every SDMA engine gets
                        # descriptors and the +16 completion sem fires
                        last = max(last, min(128, bnd - a))
                        vals[a + last : bnd] = -1
                i_in = np.arange(ncall * 128)
                idx_flat[(base16 + i_in // 16) * 16 + (i_in % 16)] = vals.astype(
                    np.int16
                )
        idx_mat = idx_flat.reshape(meta["idxcols"], 16).T  # [16, idxcols]
        idx_mat = np.tile(idx_mat, (8, 1))  # replicate to 128 partitions
        per_core.append(dict(idx=np.ascontiguousarray(idx_mat), pmat=pmat))

    return meta, per_core


def prep_inputs(cfg: Cfg, inputs):
    """Shard + lay out all per-core input tensors."""
    f = inputs["features"].astype(np.float32)
    meta, per_edge = edge_structure(
        cfg,
        inputs["edge_row"].astype(np.int64),
        inputs["edge_col"].astype(np.int64),
        inputs["edge_weight"].astype(np.float32),
    )
    kin = cfg.in_dim // 128
    k1 = cfg.h1 // 128
    k2 = cfg.h2 // 128

    def wlayout(w, kt):
        # [K, M] -> [128, kt*M] with [:, i*M:(i+1)*M] = w[i*128:(i+1)*128]
        K, M = w.shape
        return (
            w.reshape(kt, 128, M).transpose(1, 0, 2).reshape(128, kt * M)
        ).astype(BF16)

    w1 = wlayout(inputs["W_lin1"].astype(np.float32), kin)
    wg1 = wlayout(inputs["W_g1"].astype(np.float32), k1)
    wg2 = wlayout(inputs["W_g2"].astype(np.float32), k2)
    wl2 = wlayout(inputs["W_lin2"].astype(np.float32), k2)
    b1 = (
        inputs["b_lin1"].astype(np.float32).reshape(kin, 128).T.copy()
    )  # [128, kin]
    bg1 = inputs["b_g1"].astype(BF16).reshape(1, cfg.h2)
    bg2 = inputs["b_g2"].astype(BF16).reshape(1, cfg.h2)
    bl2 = inputs["b_lin2"].astype(BF16).reshape(1, cfg.out_dim)

    npc, npad = cfg.nodes_per_core, cfg.npad
    in_maps = []
    for c in range(cfg.n_cores):
        lo = c * npc
        hi = min((c + 1) * npc, cfg.n_nodes)
        xc = np.zeros((npad, cfg.in_dim), np.float32)
        xc[: hi - lo] = f[lo:hi]
        # XT layout [128, kin*npad]: [:, i*npad:(i+1)*npad] = x.T[i*128:...]
        xt = (
            xc.T.reshape(kin, 128, npad)
            .transpose(1, 0, 2)
            .reshape(128, kin * npad)
        ).astype(BF16)
        in_maps.append(
            {
                "xt": np.ascontiguousarray(xt),
                "w1": w1,
                "wg1": wg1,
                "wg2": wg2,
                "wl2": wl2,
                "b1": b1,
                "bg1": bg1,
                "bg2": bg2,
                "bl2": bl2,
                "idx": per_edge[c]["idx"],
                "pmat": per_edge[c]["pmat"],
            }
        )
    return meta, in_maps


# ---------------------------------------------------------------- kernel IR


def build(cfg: Cfg, meta):
    nc = bacc.Bacc(
        "TRN2",
        target_bir_lowering=False,
        debug=False,
        num_devices=cfg.n_cores,
        num_swdge_queues=4,
    )
    bf = mybir.dt.bfloat16
    f8 = mybir.dt.float8e4
    f32 = mybir.dt.float32
    i16 = mybir.dt.int16
    kin = cfg.in_dim // 128
    k1 = cfg.h1 // 128
    k2 = cfg.h2 // 128
    npad, nb, G, H2, OUT = (
        cfg.npad,
        cfg.nblocks,
        cfg.group_blocks,
        cfg.h2,
        cfg.out_dim,
    )
    HALF = cfg.half
    totch = meta["totch"]

    xt_d = nc.dram_tensor("xt", [128, kin * npad], bf, kind="ExternalInput").ap()
    w1_d = nc.dram_tensor("w1", [128, kin * cfg.h1], bf, kind="ExternalInput").ap()
    wg1_d = nc.dram_tensor("wg1", [128, k1 * H2], bf, kind="ExternalInput").ap()
    wg2_d = nc.dram_tensor("wg2", [128, k2 * H2], bf, kind="ExternalInput").ap()
    wl2_d = nc.dram_tensor("wl2", [128, k2 * OUT], bf, kind="ExternalInput").ap()
    b1_d = nc.dram_tensor("b1", [128, kin], f32, kind="ExternalInput").ap()
    bg1_d = nc.dram_tensor("bg1", [1, H2], bf, kind="ExternalInput").ap()
    bg2_d = nc.dram_tensor("bg2", [1, H2], bf, kind="ExternalInput").ap()
    bl2_d = nc.dram_tensor("bl2", [1, OUT], bf, kind="ExternalInput").ap()
    idx_d = nc.dram_tensor(
        "idx", [128, meta["idxcols"]], i16, kind="ExternalInput"
    ).ap()
    pmat_d = nc.dram_tensor(
        "pmat", [128, totch * 128], f8, kind="ExternalInput"
    ).ap()
    y_d = nc.dram_tensor("y", [npad, OUT], f32, kind="ExternalOutput").ap()

    hh = ((nb + 1) // 2) * 128  # A-half rows per core (block aligned)
    hB = npad - hh
    nbA = hh // 128  # blocks in A half
    HA = cfg.n_cores * hh
    HB = cfg.n_cores * hB

    g1_localA = nc.dram_tensor("g1_localA", [hh, H2], f8).ap()
    g1_localB = nc.dram_tensor("g1_localB", [hB, H2], f8).ap()
    g2_localA = nc.dram_tensor("g2_localA", [hh, H2], f8).ap()
    g2_localB = nc.dram_tensor("g2_localB", [hB, H2], f8).ap()
    g1_tableA = nc.dram_tensor("g1_tableA", [HA, H2], f8, addr_space="Shared").ap()
    g1_tableB = nc.dram_tensor("g1_tableB", [HB, H2], f8, addr_space="Shared").ap()
    g2_tableA = nc.dram_tensor("g2_tableA", [HA, H2], f8, addr_space="Shared").ap()
    g2_tableB = nc.dram_tensor("g2_tableB", [HB, H2], f8, addr_space="Shared").ap()

    rg = [list(range(cfg.n_cores))]

    def spmm(tc, ctx, nc, tables, idx_s, ones_t, brow, psum_tag, out_cb,
             after_group=None):
        """Weighted segment-sum of gathered table rows, per row-block.

        h0 gathers are issued S groups ahead of h1 so the first h1 gather
        (which waits for the B-half AllGather) doesn't starve the pipeline.
        """
        S = 5
        gp = [
            ctx.enter_context(
                tc.tile_pool(name=f"gath{psum_tag}{h}", bufs=(S + 2 if h == 0 else 4))
            )
            for h in (0, 1)
        ]
        pp = ctx.enter_context(tc.tile_pool(name=f"pm{psum_tag}", bufs=4))
        sp = ctx.enter_context(
            tc.tile_pool(name=f"ps{psum_tag}", bufs=2 * G, space="PSUM")
        )
        qstate = [0]

        def issue(g, h):
            n = meta["nch"][g][h]
            if n == 0:
                return None
            t = gp[h].tile([128, n, H2], f8, tag=f"g{h}")
            # split into <=15-chunk (1920-idx) sub-calls: a single
            # dma_gather must fit the SWDGE descriptor ring. Rotate
            # across the 4 SWDGE queues so descriptor generation runs
            # on all 4 Q7 core pairs concurrently.
            for lo in range(0, n, 15):
                ns = min(15, n - lo)
                o16 = meta["off16"][g][h] + lo * 8
                nc.gpsimd.dma_gather(
                    out_ap=t[:, lo : lo + ns, :],
                    in_ap=tables[h][:, :],
                    idxs_ap=idx_s[:, o16 : o16 + ns * 8],
                    num_idxs=ns * 128,
                    num_idxs_reg=ns * 128,
                    elem_size=H2,
                    single_packet=False,
                    queue_num=qstate[0] % 4,
                )
                qstate[0] += 1
            return t

        pend0 = {}
        for g in range(min(S, cfg.ngroups)):
            pend0[g] = issue(g, 0)
        j0 = 0
        for g in range(cfg.ngroups):
            blocks = list(range(g * G, min((g + 1) * G, nb)))
            gt = {0: pend0.pop(g)}
            if g + S < cfg.ngroups:
                pend0[g + S] = issue(g + S, 0)
            gt[1] = issue(g, 1)
            chg = meta["nch"][g][0] + meta["nch"][g][1]
            if chg > 0:
                ptile = pp.tile([128, chg * 128], f8, tag="p")
                nc.sync.dma_start(
                    ptile[:], pmat_d[:, j0 * 128 : (j0 + chg) * 128]
                )
            psums = {
                b: sp.tile([128, H2], f32, tag="ps", name=f"ps{psum_tag}_{b}")
                for b in blocks
            }
            started = dict.fromkeys(blocks, False)
            jj = 0
            for h in (0, 1):
                for jh, b in enumerate(meta["chunk_blk"][g][h]):
                    nc.tensor.matmul(
                        psums[b][:],
                        lhsT=ptile[:, jj * 128 : (jj + 1) * 128],
                        rhs=gt[h][:, jh, :],
                        start=not started[b],
                        stop=False,
                    )
                    started[b] = True
                    jj += 1
            for b in blocks:
                nc.tensor.matmul(
                    psums[b][:],
                    lhsT=ones_t[:1, :],
                    rhs=brow[:1, :],
                    start=not started[b],
                    stop=True,
                )
                out_cb(b, psums[b])
            if after_group is not None:
                after_group(g, blocks)
            j0 += chg

    with tile.TileContext(nc) as tc:
        with ExitStack() as top:
            const = top.enter_context(tc.tile_pool(name="const", bufs=1))
            w1_s = const.tile([128, kin * cfg.h1], bf)
            nc.sync.dma_start(w1_s[:], w1_d[:, :])
            wg1_s = const.tile([128, k1 * H2], bf)
            nc.sync.dma_start(wg1_s[:], wg1_d[:, :])
            wg2_s = const.tile([128, k2 * H2], bf)
            nc.sync.dma_start(wg2_s[:], wg2_d[:, :])
            wl2_s = const.tile([128, k2 * OUT], bf)
            nc.sync.dma_start(wl2_s[:], wl2_d[:, :])
            b1_s = const.tile([128, kin], f32)
            nc.sync.dma_start(b1_s[:], b1_d[:, :])
            bg1_s = const.tile([1, H2], bf)
            nc.sync.dma_start(bg1_s[:], bg1_d[:, :])
            bg2_s = const.tile([1, H2], bf)
            nc.sync.dma_start(bg2_s[:], bg2_d[:, :])
            bl2_s = const.tile([1, OUT], bf)
            nc.sync.dma_start(bl2_s[:], bl2_d[:, :])
            idx_s = const.tile([128, meta["idxcols"]], i16)
            nc.sync.dma_start(idx_s[:], idx_d[:, :])
            ident = const.tile([128, 128], bf)
            make_identity(nc, ident[:])
            ones_t = const.tile([1, 128], bf)
            nc.gpsimd.memset(ones_t[:], 1.0)

            # ---------------- L1: h1T[f, n] = sigmoid(W1.T @ X.T + b1)
            with ExitStack() as ph1:
                h1p = ph1.enter_context(tc.tile_pool(name="h1t", bufs=1))
                h1t = h1p.tile([128, k1 * npad], bf)
                with ExitStack() as px:
                    xp = px.enter_context(tc.tile_pool(name="xt", bufs=1))
                    psp = px.enter_context(
                        tc.tile_pool(name="ps1", bufs=4, space="PSUM")
                    )
                    xt_k = []
                    for kt in range(kin):
                        xk = xp.tile([128, npad], bf, name=f"xt{kt}")
                        nc.sync.dma_start(
                            xk[:], xt_d[:, kt * npad : (kt + 1) * npad]
                        )
                        xt_k.append(xk)
                    nsl = [(i * 512, min((i + 1) * 512, npad)) for i in range(math.ceil(npad / 512))]
                    for f1t in range(k1):
                        for a, b_ in nsl:
                            nw = b_ - a
                            ps = psp.tile([128, 512], f32, tag="ps")
                            for kt in range(kin):
                                nc.tensor.matmul(
                                    ps[:, :nw],
                                    lhsT=w1_s[
                                        :,
                                        kt * cfg.h1
                                        + f1t * 128 : kt * cfg.h1
                                        + f1t * 128
                                        + 128,
                                    ],
                                    rhs=xt_k[kt][:, a:b_],
                                    start=(kt == 0),
                                    stop=(kt == kin - 1),
                                )
                            nc.scalar.activation(
                                h1t[:, f1t * npad + a : f1t * npad + b_],
                                ps[:, :nw],
                                AF.Sigmoid,
                                bias=b1_s[:, f1t : f1t + 1],
                            )

                # ---------------- L2a: g1[n, h2] = h1 @ Wg1  (lhsT = h1T)
                def store_half(local_a, local_b, b, tile_):
                    if b < nbA:
                        nc.sync.dma_start(
                            local_a[b * 128 : (b + 1) * 128, :], tile_[:]
                        )
                    else:
                        bb = b - nbA
                        nc.sync.dma_start(
                            local_b[bb * 128 : (bb + 1) * 128, :], tile_[:]
                        )

                def allgather(ins_, outs_):
                    nc.gpsimd.collective_compute(
                        "AllGather",
                        mybir.AluOpType.bypass,
                        replica_groups=rg,
                        ins=[ins_],
                        outs=[outs_],
                    )

                with ExitStack() as p2:
                    psp2 = p2.enter_context(
                        tc.tile_pool(name="ps2", bufs=4, space="PSUM")
                    )
                    tp2 = p2.enter_context(tc.tile_pool(name="g1t", bufs=3))
                    for b in range(nb):
                        ps = psp2.tile([128, H2], f32, tag="ps")
                        for kt in range(k1):
                            nc.tensor.matmul(
                                ps[:],
                                lhsT=h1t[
                                    :, kt * npad + b * 128 : kt * npad + b * 128 + 128
                                ],
                                rhs=wg1_s[:, kt * H2 : (kt + 1) * H2],
                                start=(kt == 0),
                                stop=(kt == k1 - 1),
                            )
                        g1tile = tp2.tile([128, H2], f8, tag="g1")
                        nc.vector.tensor_copy(g1tile[:], ps[:])
                        store_half(g1_localA, g1_localB, b, g1tile)
                        if b == nbA - 1:
                            allgather(g1_localA[:, :], g1_tableA[:, :])
                    allgather(g1_localB[:, :], g1_tableB[:, :])

            # ---------------- spmm1 -> h2, L3a (g2) fused per block,
            # AG2 halves issued as soon as their blocks are stored
            with ExitStack() as ph2:
                h2p = ph2.enter_context(tc.tile_pool(name="h2res", bufs=1))
                h2r = h2p.tile([128, nb * H2], bf)
                tps = ph2.enter_context(
                    tc.tile_pool(name="tps", bufs=1, space="PSUM")
                )
                psp3 = ph2.enter_context(
                    tc.tile_pool(name="ps3", bufs=1, space="PSUM")
                )
                tp3 = ph2.enter_context(tc.tile_pool(name="l3t", bufs=4))

                with ExitStack() as ps1:
                    def cb1(b, psum):
                        nc.scalar.activation(
                            h2r[:, b * H2 : (b + 1) * H2], psum[:], AF.Relu
                        )
                        h2T = tp3.tile([128, k2, 128], bf, tag="h2T")
                        for kt in range(k2):
                            pt = tps.tile([128, 128], bf, tag="pt")
                            nc.tensor.transpose(
                                pt[:],
                                h2r[:, b * H2 + kt * 128 : b * H2 + (kt + 1) * 128],
                                ident[:],
                            )
                            nc.vector.tensor_copy(h2T[:, kt, :], pt[:])
                        ps = psp3.tile([128, H2], f32, tag="ps")
                        for kt in range(k2):
                            nc.tensor.matmul(
                                ps[:],
                                lhsT=h2T[:, kt, :],
                                rhs=wg2_s[:, kt * H2 : (kt + 1) * H2],
                                start=(kt == 0),
                                stop=(kt == k2 - 1),
                            )
                        g2tile = tp3.tile([128, H2], f8, tag="g2")
                        nc.vector.tensor_copy(g2tile[:], ps[:])
                        store_half(g2_localA, g2_localB, b, g2tile)

                    spmm(tc, ps1, nc, (g1_tableA, g1_tableB), idx_s, ones_t,
                         bg1_s, "a", cb1)

                # both AG2 halves after the full spmm1/L3a stream: a
                # mid-stream collective trigger blocks the gpsimd queue on
                # cross-core rendezvous and stalls the gather pipeline
                allgather(g2_localA[:, :], g2_tableA[:, :])
                allgather(g2_localB[:, :], g2_tableB[:, :])

            # ---------------- spmm2 + L4 fused per block
            with ExitStack() as ps2x:
                tps4 = ps2x.enter_context(
                    tc.tile_pool(name="tps4", bufs=1, space="PSUM")
                )
                psp4 = ps2x.enter_context(
                    tc.tile_pool(name="ps4", bufs=1, space="PSUM")
                )
                tp4 = ps2x.enter_context(tc.tile_pool(name="l4t", bufs=3))

                def cb2(b, psum):
                    h3t = tp4.tile([128, H2], bf, tag="h3")
                    nc.scalar.activation(h3t[:], psum[:], AF.Relu)
                    h3T = tp4.tile([128, k2, 128], bf, tag="h3T")
                    for kt in range(k2):
                        pt = tps4.tile([128, 128], bf, tag="pt")
                        nc.tensor.transpose(
                            pt[:], h3t[:, kt * 128 : (kt + 1) * 128], ident[:]
                        )
                        nc.vector.tensor_copy(h3T[:, kt, :], pt[:])
                    ps4 = psp4.tile([128, OUT], f32, tag="ps")
                    for kt in range(k2):
                        nc.tensor.matmul(
                            ps4[:],
                            lhsT=h3T[:, kt, :],
                            rhs=wl2_s[:, kt * OUT : (kt + 1) * OUT],
                            start=(kt == 0),
                            stop=False,
                        )
                    nc.tensor.matmul(
                        ps4[:],
                        lhsT=ones_t[:1, :],
                        rhs=bl2_s[:1, :],
                        start=False,
                        stop=True,
                    )
                    yt = tp4.tile([128, OUT], f32, tag="y")
                    nc.vector.tensor_copy(yt[:], ps4[:])
                    nc.sync.dma_start(y_d[b * 128 : (b + 1) * 128, :], yt[:])

                spmm(tc, ps2x, nc, (g2_tableA, g2_tableB), idx_s, ones_t,
                     bg2_s, "b", cb2)

    nc.compile()
    return nc


# ---------------------------------------------------------------- driver

_CACHE = {}


def run(inputs, cfg: Cfg = FULL, trace=False, tmpdir=None):
    meta, in_maps = prep_inputs(cfg, inputs)
    key = (cfg, meta["totch"], meta["idxcols"])
    if key not in _CACHE:
        _CACHE[key] = build(cfg, meta)
    nc = _CACHE[key]
    res = run_bass_kernel_spmd(
        nc,
        in_maps,
        core_ids=list(range(cfg.n_cores)),
        trace=trace,
        tmpdir=tmpdir,
    )
    npc = cfg.nodes_per_core
    out = np.empty((cfg.n_nodes, cfg.out_dim), np.float32)
    for c in range(cfg.n_cores):
        lo = c * npc
        hi = min((c + 1) * npc, cfg.n_nodes)
        out[lo:hi] = res.results[c]["y"][: hi - lo]
    return out, res


def kernel(**inputs) -> np.ndarray:
    out, _ = run(inputs, FULL, trace=False)
    return out



# revision 35
# speedup vs baseline: 1.0890x; 1.0042x over previous
"""GNN message-passing kernel for 8 Trainium2 NeuronCores (Bass/Tile).

Takes FULL inputs, shards nodes across 8 cores internally, runs the
4-layer GNN (dense -> spmm -> spmm -> dense) with two bf16 AllGathers
of the hidden node table, and PE-matmul-based weighted segment sums
(host-built one-hot selector matrices), then gathers the full output.
"""

import math
from contextlib import ExitStack
from dataclasses import dataclass

import ml_dtypes
import numpy as np

import concourse.bass as bass
import concourse.mybir as mybir
import concourse.tile as tile
from concourse import bacc
from concourse.bass_utils import run_bass_kernel_spmd
from concourse.masks import make_identity

BF16 = ml_dtypes.bfloat16
FP8 = ml_dtypes.float8_e4m3fn
AF = mybir.ActivationFunctionType


@dataclass(frozen=True)
class Cfg:
    n_nodes: int = 50000
    n_edges: int = 800000
    in_dim: int = 512
    h1: int = 512
    h2: int = 256
    out_dim: int = 128
    n_cores: int = 8
    group_blocks: int = 3  # row-blocks per gather group

    @property
    def nodes_per_core(self):
        return math.ceil(self.n_nodes / self.n_cores)

    @property
    def npad(self):  # per-core padded nodes
        return math.ceil(self.nodes_per_core / 128) * 128

    @property
    def nblocks(self):
        return self.npad // 128

    @property
    def ntot(self):
        return self.npad * self.n_cores

    @property
    def half(self):
        return self.ntot // 2

    @property
    def ngroups(self):
        return math.ceil(self.nblocks / self.group_blocks)


FULL = Cfg()


# ---------------------------------------------------------------- host prep


def edge_structure(cfg: Cfg, edge_row, edge_col, edge_weight):
    """Per-core edge streams with SPMD-uniform chunk counts.

    Returns (meta, per_core) where meta has the uniform chunk structure:
      meta['nch'][g][h]      total chunks in gather call (group g, half h)
      meta['chunk_blk'][g][h] list of block ids (one per chunk, ordered)
      meta['off16'][g][h]    idx-tile column offset (int16 cols) of the call
      meta['totch']          total chunks
      meta['idxcols']        total int16 columns of the idx tensor
    per_core[c] = dict(idx=[128, idxcols] int16, pmat=[128, totch*128] bf16)
    """
    nc_, npad, half, nb, G = (
        cfg.n_cores,
        cfg.npad,
        cfg.half,
        cfg.nblocks,
        cfg.group_blocks,
    )
    npc = cfg.nodes_per_core
    assert half <= 32767, "half-table must fit int16 indices"

    core_of = edge_row // npc
    lr_all = edge_row - core_of * npc  # local row
    # Block-aligned sub-table split: half 0 (A) holds every core's first
    # ceil(nb/2) row-blocks, half 1 (B) the rest, so table-half A can be
    # AllGathered as soon as each core finishes its first 25 blocks.
    # cl = position within the half-table.
    hh = ((nb + 1) // 2) * 128
    hB = npad - hh
    assert nc_ * hh <= 32767
    core_col = edge_col // npc
    loc = edge_col - core_col * npc
    half_all = (loc >= hh).astype(np.int64)
    cl_all = np.where(half_all == 0, core_col * hh + loc,
                      core_col * hB + (loc - hh))

    # bucket edges per (core, block, half)
    per = {}
    counts = np.zeros((nc_, nb, 2), np.int64)
    for c in range(nc_):
        m = core_of == c
        lr, cl, hf, w = lr_all[m], cl_all[m], half_all[m], edge_weight[m]
        blk = lr // 128
        order = np.lexsort((lr, hf, blk))
        per[c] = (lr[order], cl[order], hf[order], w[order], blk[order])
        np.add.at(counts[c], (blk, hf), 1)

    # uniform chunks per (block, half) = max over cores
    chunks_bh = np.ceil(counts / 128.0).astype(np.int64).max(axis=0)  # [nb, 2]

    ngroups = cfg.ngroups
    nch = [[0, 0] for _ in range(ngroups)]
    chunk_blk = [[[], []] for _ in range(ngroups)]
    off16 = [[0, 0] for _ in range(ngroups)]
    tot16 = 0
    totch = 0
    for g in range(ngroups):
        blocks = range(g * G, min((g + 1) * G, nb))
        for h in (0, 1):
            off16[g][h] = tot16
            n = 0
            for b in blocks:
                cb = int(chunks_bh[b, h])
                chunk_blk[g][h].extend([b] * cb)
                n += cb
            nch[g][h] = n
            tot16 += n * 8  # 128 idx per chunk -> 8 int16 cols
            totch += n

    meta = dict(
        nch=nch,
        chunk_blk=chunk_blk,
        off16=off16,
        totch=totch,
        idxcols=max(tot16, 8),
        chunks_bh=chunks_bh,
    )

    # SPMD-uniform chunk order within each (group, half): ascending
    # chunk-index-within-block (j), block-major within each j. Low-j chunks
    # are full for every core; high-j chunks carry the pad for every core,
    # so the pad concentrates at the call tail uniformly.
    # chunk_blk is rebuilt to match this order.
    for g in range(ngroups):
        blocks = list(range(g * G, min((g + 1) * G, nb)))
        for h in (0, 1):
            order = []
            maxcb = max((int(chunks_bh[b, h]) for b in blocks), default=0)
            for j in range(maxcb):
                for b in blocks:
                    if j < int(chunks_bh[b, h]):
                        order.append((b, j))
            meta["chunk_blk"][g][h] = [b for b, _ in order]
            meta.setdefault("chunk_ord", {})[(g, h)] = order

    # Pad slots: idx=-1 only in the TRAILING run of each <=15-chunk
    # sub-call (the gather ucode trims trailing negatives before
    # descriptor emission — saves Q7 time and DMA). Mid-call pad and the
    # first SAFE_GROUPS groups (whose SBUF tiles are uninitialized and
    # could hold NaN bytes; later groups reuse tiles holding valid old
    # values) use idx=0: gathers row 0, killed by zero pmat weight.
    SAFE_GROUPS = 999  # -1 trim disabled: trimmed calls hang the DMA sem protocol
    per_core = []
    for c in range(nc_):
        lr, cl, hf, w, blk = per[c]
        idx_flat = np.zeros(meta["idxcols"] * 16, np.int16)
        pmat = np.zeros((128, totch * 128), FP8)
        jchunk = 0
        for g in range(ngroups):
            blocks = list(range(g * G, min((g + 1) * G, nb)))
            for h in (0, 1):
                bdata = {}
                for b in blocks:
                    sel = (blk == b) & (hf == h)
                    bdata[b] = (cl[sel], lr[sel], w[sel])
                order = meta["chunk_ord"][(g, h)]
                ncall = len(order)
                base16 = meta["off16"][g][h]
                vals = np.zeros(ncall * 128, np.int64)
                real = np.zeros(ncall * 128, bool)
                for jj, (b, j) in enumerate(order):
                    e_cl, e_lr, e_w = bdata[b]
                    lo, hi = j * 128, min(j * 128 + 128, len(e_cl))
                    nreal = max(hi - lo, 0)
                    if nreal > 0:
                        vals[jj * 128 : jj * 128 + nreal] = e_cl[lo:hi]
                        real[jj * 128 : jj * 128 + nreal] = True
                        slot = np.arange(nreal)
                        r = e_lr[lo:hi] - b * 128
                        pmat[slot, (jchunk + jj) * 128 + r] = e_w[lo:hi].astype(
                            FP8
                        )
                jchunk += ncall
                if g >= SAFE_GROUPS:
                    # trailing trim per sub-call segment of 15 chunks
                    for seg in range(0, ncall, 15):
                        a, bnd = seg * 128, min(seg + 15, ncall) * 128
                        rseg = real[a:bnd]
                        nz = np.nonzero(rseg)[0]
                        last = nz[-1] + 1 if len(nz) else 0
                        # keep >=128 idx so every SDMA engine gets
                        # descriptors and the +16 completion sem fires
                        last = max(last, min(128, bnd - a))
                        vals[a + last : bnd] = -1
                i_in = np.arange(ncall * 128)
                idx_flat[(base16 + i_in // 16) * 16 + (i_in % 16)] = vals.astype(
                    np.int16
                )
        idx_mat = idx_flat.reshape(meta["idxcols"], 16).T  # [16, idxcols]
        idx_mat = np.tile(idx_mat, (8, 1))  # replicate to 128 partitions
        per_core.append(dict(idx=np.ascontiguousarray(idx_mat), pmat=pmat))

    return meta, per_core


def prep_inputs(cfg: Cfg, inputs):
    """Shard + lay out all per-core input tensors."""
    f = inputs["features"].astype(np.float32)
    meta, per_edge = edge_structure(
        cfg,
        inputs["edge_row"].astype(np.int64),
        inputs["edge_col"].astype(np.int64),
        inputs["edge_weight"].astype(np.float32),
    )
    kin = cfg.in_dim // 128
    k1 = cfg.h1 // 128
    k2 = cfg.h2 // 128

    def wlayout(w, kt):
        # [K, M] -> [128, kt*M] with [:, i*M:(i+1)*M] = w[i*128:(i+1)*128]
        K, M = w.shape
        return (
            w.reshape(kt, 128, M).transpose(1, 0, 2).reshape(128, kt * M)
        ).astype(BF16)

    w1 = wlayout(inputs["W_lin1"].astype(np.float32), kin)
    wg1 = wlayout(inputs["W_g1"].astype(np.float32), k1)
    wg2 = wlayout(inputs["W_g2"].astype(np.float32), k2)
    wl2 = wlayout(inputs["W_lin2"].astype(np.float32), k2)
    b1 = (
        inputs["b_lin1"].astype(np.float32).reshape(kin, 128).T.copy()
    )  # [128, kin]
    bg1 = np.tile(inputs["b_g1"].astype(np.float32).reshape(1, cfg.h2), (128, 1))
    bg2 = np.tile(inputs["b_g2"].astype(np.float32).reshape(1, cfg.h2), (128, 1))
    bl2 = np.tile(inputs["b_lin2"].astype(np.float32).reshape(1, cfg.out_dim), (128, 1))

    npc, npad = cfg.nodes_per_core, cfg.npad
    in_maps = []
    for c in range(cfg.n_cores):
        lo = c * npc
        hi = min((c + 1) * npc, cfg.n_nodes)
        xc = np.zeros((npad, cfg.in_dim), np.float32)
        xc[: hi - lo] = f[lo:hi]
        # XT layout [128, kin*npad]: [:, i*npad:(i+1)*npad] = x.T[i*128:...]
        xt = (
            xc.T.reshape(kin, 128, npad)
            .transpose(1, 0, 2)
            .reshape(128, kin * npad)
        ).astype(BF16)
        in_maps.append(
            {
                "xt": np.ascontiguousarray(xt),
                "w1": w1,
                "wg1": wg1,
                "wg2": wg2,
                "wl2": wl2,
                "b1": b1,
                "bg1": bg1,
                "bg2": bg2,
                "bl2": bl2,
                "idx": per_edge[c]["idx"],
                "pmat": per_edge[c]["pmat"],
            }
        )
    return meta, in_maps


# ---------------------------------------------------------------- kernel IR


def build(cfg: Cfg, meta):
    nc = bacc.Bacc(
        "TRN2",
        target_bir_lowering=False,
        debug=False,
        num_devices=cfg.n_cores,
        num_swdge_queues=4,
    )
    bf = mybir.dt.bfloat16
    f8 = mybir.dt.float8e4
    f32 = mybir.dt.float32
    i16 = mybir.dt.int16
    kin = cfg.in_dim // 128
    k1 = cfg.h1 // 128
    k2 = cfg.h2 // 128
    npad, nb, G, H2, OUT = (
        cfg.npad,
        cfg.nblocks,
        cfg.group_blocks,
        cfg.h2,
        cfg.out_dim,
    )
    HALF = cfg.half
    totch = meta["totch"]

    xt_d = nc.dram_tensor("xt", [128, kin * npad], bf, kind="ExternalInput").ap()
    w1_d = nc.dram_tensor("w1", [128, kin * cfg.h1], bf, kind="ExternalInput").ap()
    wg1_d = nc.dram_tensor("wg1", [128, k1 * H2], bf, kind="ExternalInput").ap()
    wg2_d = nc.dram_tensor("wg2", [128, k2 * H2], bf, kind="ExternalInput").ap()
    wl2_d = nc.dram_tensor("wl2", [128, k2 * OUT], bf, kind="ExternalInput").ap()
    b1_d = nc.dram_tensor("b1", [128, kin], f32, kind="ExternalInput").ap()
    bg1_d = nc.dram_tensor("bg1", [128, H2], f32, kind="ExternalInput").ap()
    bg2_d = nc.dram_tensor("bg2", [128, H2], f32, kind="ExternalInput").ap()
    bl2_d = nc.dram_tensor("bl2", [128, OUT], f32, kind="ExternalInput").ap()
    idx_d = nc.dram_tensor(
        "idx", [128, meta["idxcols"]], i16, kind="ExternalInput"
    ).ap()
    pmat_d = nc.dram_tensor(
        "pmat", [128, totch * 128], f8, kind="ExternalInput"
    ).ap()
    y_d = nc.dram_tensor("y", [npad, OUT], f32, kind="ExternalOutput").ap()

    hh = ((nb + 1) // 2) * 128  # A-half rows per core (block aligned)
    hB = npad - hh
    nbA = hh // 128  # blocks in A half
    HA = cfg.n_cores * hh
    HB = cfg.n_cores * hB

    g1_localA = nc.dram_tensor("g1_localA", [hh, H2], f8).ap()
    g1_localB = nc.dram_tensor("g1_localB", [hB, H2], f8).ap()
    g2_localA = nc.dram_tensor("g2_localA", [hh, H2], f8).ap()
    g2_localB = nc.dram_tensor("g2_localB", [hB, H2], f8).ap()
    g1_tableA = nc.dram_tensor("g1_tableA", [HA, H2], f8, addr_space="Shared").ap()
    g1_tableB = nc.dram_tensor("g1_tableB", [HB, H2], f8, addr_space="Shared").ap()
    g2_tableA = nc.dram_tensor("g2_tableA", [HA, H2], f8, addr_space="Shared").ap()
    g2_tableB = nc.dram_tensor("g2_tableB", [HB, H2], f8, addr_space="Shared").ap()

    rg = [list(range(cfg.n_cores))]

    def spmm(tc, ctx, nc, tables, idx_s, ones_t, brow, psum_tag, out_cb,
             after_group=None):
        """Weighted segment-sum of gathered table rows, per row-block.

        h0 gathers are issued S groups ahead of h1 so the first h1 gather
        (which waits for the B-half AllGather) doesn't starve the pipeline.
        """
        S = 5
        gp = [
            ctx.enter_context(
                tc.tile_pool(name=f"gath{psum_tag}{h}", bufs=(S + 2 if h == 0 else 4))
            )
            for h in (0, 1)
        ]
        pp = ctx.enter_context(tc.tile_pool(name=f"pm{psum_tag}", bufs=4))
        sp = ctx.enter_context(
            tc.tile_pool(name=f"ps{psum_tag}", bufs=2 * G, space="PSUM")
        )
        qstate = [0]

        def issue(g, h):
            n = meta["nch"][g][h]
            if n == 0:
                return None
            t = gp[h].tile([128, n, H2], f8, tag=f"g{h}")
            # split into <=15-chunk (1920-idx) sub-calls: a single
            # dma_gather must fit the SWDGE descriptor ring. Rotate
            # across the 4 SWDGE queues so descriptor generation runs
            # on all 4 Q7 core pairs concurrently.
            for lo in range(0, n, 15):
                ns = min(15, n - lo)
                o16 = meta["off16"][g][h] + lo * 8
                nc.gpsimd.dma_gather(
                    out_ap=t[:, lo : lo + ns, :],
                    in_ap=tables[h][:, :],
                    idxs_ap=idx_s[:, o16 : o16 + ns * 8],
                    num_idxs=ns * 128,
                    num_idxs_reg=ns * 128,
                    elem_size=H2,
                    single_packet=False,
                    queue_num=qstate[0] % 4,
                )
                qstate[0] += 1
            return t

        pend0 = {}
        for g in range(min(S, cfg.ngroups)):
            pend0[g] = issue(g, 0)
        j0 = 0
        for g in range(cfg.ngroups):
            blocks = list(range(g * G, min((g + 1) * G, nb)))
            gt = {0: pend0.pop(g)}
            if g + S < cfg.ngroups:
                pend0[g + S] = issue(g + S, 0)
            gt[1] = issue(g, 1)
            chg = meta["nch"][g][0] + meta["nch"][g][1]
            if chg > 0:
                ptile = pp.tile([128, chg * 128], f8, tag="p")
                nc.sync.dma_start(
                    ptile[:], pmat_d[:, j0 * 128 : (j0 + chg) * 128]
                )
            psums = {
                b: sp.tile([128, H2], f32, tag="ps", name=f"ps{psum_tag}_{b}")
                for b in blocks
            }
            last_jj = {}
            jj = 0
            for h in (0, 1):
                for b in meta["chunk_blk"][g][h]:
                    last_jj[b] = jj
                    jj += 1
            started = dict.fromkeys(blocks, False)
            jj = 0
            for h in (0, 1):
                for jh, b in enumerate(meta["chunk_blk"][g][h]):
                    nc.tensor.matmul(
                        psums[b][:],
                        lhsT=ptile[:, jj * 128 : (jj + 1) * 128],
                        rhs=gt[h][:, jh, :],
                        start=not started[b],
                        stop=(jj == last_jj[b]),
                    )
                    started[b] = True
                    jj += 1
            for b in blocks:
                assert started[b], f"block {b} has no chunks"
                out_cb(b, psums[b])
            if after_group is not None:
                after_group(g, blocks)
            j0 += chg

    with tile.TileContext(nc) as tc:
        with ExitStack() as top:
            const = top.enter_context(tc.tile_pool(name="const", bufs=1))
            w1_s = const.tile([128, kin * cfg.h1], bf)
            nc.sync.dma_start(w1_s[:], w1_d[:, :])
            wg1_s = const.tile([128, k1 * H2], bf)
            nc.sync.dma_start(wg1_s[:], wg1_d[:, :])
            wg2_s = const.tile([128, k2 * H2], bf)
            nc.sync.dma_start(wg2_s[:], wg2_d[:, :])
            wl2_s = const.tile([128, k2 * OUT], bf)
            nc.sync.dma_start(wl2_s[:], wl2_d[:, :])
            b1_s = const.tile([128, kin], f32)
            nc.sync.dma_start(b1_s[:], b1_d[:, :])
            bg1_s = const.tile([128, H2], f32)
            nc.sync.dma_start(bg1_s[:], bg1_d[:, :])
            bg2_s = const.tile([128, H2], f32)
            nc.sync.dma_start(bg2_s[:], bg2_d[:, :])
            bl2_s = const.tile([128, OUT], f32)
            nc.sync.dma_start(bl2_s[:], bl2_d[:, :])
            idx_s = const.tile([128, meta["idxcols"]], i16)
            nc.sync.dma_start(idx_s[:], idx_d[:, :])
            ident = const.tile([128, 128], bf)
            make_identity(nc, ident[:])
            ones_t = const.tile([1, 128], bf)
            nc.gpsimd.memset(ones_t[:], 1.0)

            # ---------------- L1: h1T[f, n] = sigmoid(W1.T @ X.T + b1)
            with ExitStack() as ph1:
                h1p = ph1.enter_context(tc.tile_pool(name="h1t", bufs=1))
                h1t = h1p.tile([128, k1 * npad], bf)
                with ExitStack() as px:
                    xp = px.enter_context(tc.tile_pool(name="xt", bufs=1))
                    psp = px.enter_context(
                        tc.tile_pool(name="ps1", bufs=4, space="PSUM")
                    )
                    xt_k = []
                    for kt in range(kin):
                        xk = xp.tile([128, npad], bf, name=f"xt{kt}")
                        nc.sync.dma_start(
                            xk[:], xt_d[:, kt * npad : (kt + 1) * npad]
                        )
                        xt_k.append(xk)
                    nsl = [(i * 512, min((i + 1) * 512, npad)) for i in range(math.ceil(npad / 512))]
                    for f1t in range(k1):
                        for a, b_ in nsl:
                            nw = b_ - a
                            ps = psp.tile([128, 512], f32, tag="ps")
                            for kt in range(kin):
                                nc.tensor.matmul(
                                    ps[:, :nw],
                                    lhsT=w1_s[
                                        :,
                                        kt * cfg.h1
                                        + f1t * 128 : kt * cfg.h1
                                        + f1t * 128
                                        + 128,
                                    ],
                                    rhs=xt_k[kt][:, a:b_],
                                    start=(kt == 0),
                                    stop=(kt == kin - 1),
                                )
                            nc.scalar.activation(
                                h1t[:, f1t * npad + a : f1t * npad + b_],
                                ps[:, :nw],
                                AF.Sigmoid,
                                bias=b1_s[:, f1t : f1t + 1],
                            )

                # ---------------- L2a: g1[n, h2] = h1 @ Wg1  (lhsT = h1T)
                def store_half(local_a, local_b, b, tile_):
                    if b < nbA:
                        nc.sync.dma_start(
                            local_a[b * 128 : (b + 1) * 128, :], tile_[:]
                        )
                    else:
                        bb = b - nbA
                        nc.sync.dma_start(
                            local_b[bb * 128 : (bb + 1) * 128, :], tile_[:]
                        )

                def allgather(ins_, outs_):
                    nc.gpsimd.collective_compute(
                        "AllGather",
                        mybir.AluOpType.bypass,
                        replica_groups=rg,
                        ins=[ins_],
                        outs=[outs_],
                    )

                with ExitStack() as p2:
                    psp2 = p2.enter_context(
                        tc.tile_pool(name="ps2", bufs=4, space="PSUM")
                    )
                    tp2 = p2.enter_context(tc.tile_pool(name="g1t", bufs=3))
                    for b in range(nb):
                        ps = psp2.tile([128, H2], f32, tag="ps")
                        for kt in range(k1):
                            nc.tensor.matmul(
                                ps[:],
                                lhsT=h1t[
                                    :, kt * npad + b * 128 : kt * npad + b * 128 + 128
                                ],
                                rhs=wg1_s[:, kt * H2 : (kt + 1) * H2],
                                start=(kt == 0),
                                stop=(kt == k1 - 1),
                            )
                        g1tile = tp2.tile([128, H2], f8, tag="g1")
                        nc.vector.tensor_copy(g1tile[:], ps[:])
                        store_half(g1_localA, g1_localB, b, g1tile)
                        if b == nbA - 1:
                            allgather(g1_localA[:, :], g1_tableA[:, :])
                    allgather(g1_localB[:, :], g1_tableB[:, :])

            # ---------------- spmm1 -> h2, L3a (g2) fused per block,
            # AG2 halves issued as soon as their blocks are stored
            with ExitStack() as ph2:
                h2p = ph2.enter_context(tc.tile_pool(name="h2res", bufs=1))
                h2r = h2p.tile([128, nb * H2], bf)
                tps = ph2.enter_context(
                    tc.tile_pool(name="tps", bufs=1, space="PSUM")
                )
                psp3 = ph2.enter_context(
                    tc.tile_pool(name="ps3", bufs=1, space="PSUM")
                )
                tp3 = ph2.enter_context(tc.tile_pool(name="l3t", bufs=4))

                with ExitStack() as ps1:
                    def cb1(b, psum):
                        tmp = tp3.tile([128, H2], f32, tag="tmp")
                        nc.vector.tensor_add(tmp[:], psum[:], bg1_s[:])
                        nc.vector.tensor_scalar_max(
                            h2r[:, b * H2 : (b + 1) * H2], tmp[:], 0.0
                        )
                        h2T = tp3.tile([128, k2, 128], bf, tag="h2T")
                        for kt in range(k2):
                            pt = tps.tile([128, 128], bf, tag="pt")
                            nc.tensor.transpose(
                                pt[:],
                                h2r[:, b * H2 + kt * 128 : b * H2 + (kt + 1) * 128],
                                ident[:],
                            )
                            nc.vector.tensor_copy(h2T[:, kt, :], pt[:])
                        ps = psp3.tile([128, H2], f32, tag="ps")
                        for kt in range(k2):
                            nc.tensor.matmul(
                                ps[:],
                                lhsT=h2T[:, kt, :],
                                rhs=wg2_s[:, kt * H2 : (kt + 1) * H2],
                                start=(kt == 0),
                                stop=(kt == k2 - 1),
                            )
                        g2tile = tp3.tile([128, H2], f8, tag="g2")
                        nc.vector.tensor_copy(g2tile[:], ps[:])
                        store_half(g2_localA, g2_localB, b, g2tile)

                    spmm(tc, ps1, nc, (g1_tableA, g1_tableB), idx_s, ones_t,
                         bg1_s, "a", cb1)

                # both AG2 halves after the full spmm1/L3a stream: a
                # mid-stream collective trigger blocks the gpsimd queue on
                # cross-core rendezvous and stalls the gather pipeline
                allgather(g2_localA[:, :], g2_tableA[:, :])
                allgather(g2_localB[:, :], g2_tableB[:, :])

            # ---------------- spmm2 + L4 fused per block
            with ExitStack() as ps2x:
                tps4 = ps2x.enter_context(
                    tc.tile_pool(name="tps4", bufs=1, space="PSUM")
                )
                psp4 = ps2x.enter_context(
                    tc.tile_pool(name="ps4", bufs=1, space="PSUM")
                )
                tp4 = ps2x.enter_context(tc.tile_pool(name="l4t", bufs=3))

                def cb2(b, psum):
                    tmp = tp4.tile([128, H2], f32, tag="tmp4")
                    nc.vector.tensor_add(tmp[:], psum[:], bg2_s[:])
                    h3t = tp4.tile([128, H2], bf, tag="h3")
                    nc.vector.tensor_scalar_max(h3t[:], tmp[:], 0.0)
                    h3T = tp4.tile([128, k2, 128], bf, tag="h3T")
                    for kt in range(k2):
                        pt = tps4.tile([128, 128], bf, tag="pt")
                        nc.tensor.transpose(
                            pt[:], h3t[:, kt * 128 : (kt + 1) * 128], ident[:]
                        )
                        nc.vector.tensor_copy(h3T[:, kt, :], pt[:])
                    ps4 = psp4.tile([128, OUT], f32, tag="ps")
                    for kt in range(k2):
                        nc.tensor.matmul(
                            ps4[:],
                            lhsT=h3T[:, kt, :],
                            rhs=wl2_s[:, kt * OUT : (kt + 1) * OUT],
                            start=(kt == 0),
                            stop=(kt == k2 - 1),
                        )
                    yt = tp4.tile([128, OUT], f32, tag="y")
                    nc.vector.tensor_add(yt[:], ps4[:], bl2_s[:])
                    nc.sync.dma_start(y_d[b * 128 : (b + 1) * 128, :], yt[:])

                spmm(tc, ps2x, nc, (g2_tableA, g2_tableB), idx_s, ones_t,
                     bg2_s, "b", cb2)

    nc.compile()
    return nc


# ---------------------------------------------------------------- driver

_CACHE = {}


def run(inputs, cfg: Cfg = FULL, trace=False, tmpdir=None):
    meta, in_maps = prep_inputs(cfg, inputs)
    key = (cfg, meta["totch"], meta["idxcols"])
    if key not in _CACHE:
        _CACHE[key] = build(cfg, meta)
    nc = _CACHE[key]
    res = run_bass_kernel_spmd(
        nc,
        in_maps,
        core_ids=list(range(cfg.n_cores)),
        trace=trace,
        tmpdir=tmpdir,
    )
    npc = cfg.nodes_per_core
    out = np.empty((cfg.n_nodes, cfg.out_dim), np.float32)
    for c in range(cfg.n_cores):
        lo = c * npc
        hi = min((c + 1) * npc, cfg.n_nodes)
        out[lo:hi] = res.results[c]["y"][: hi - lo]
    return out, res


def kernel(**inputs) -> np.ndarray:
    out, _ = run(inputs, FULL, trace=False)
    return out



# revision 36
# speedup vs baseline: 1.0935x; 1.0042x over previous
"""GNN message-passing kernel for 8 Trainium2 NeuronCores (Bass/Tile).

Takes FULL inputs, shards nodes across 8 cores internally, runs the
4-layer GNN (dense -> spmm -> spmm -> dense) with two bf16 AllGathers
of the hidden node table, and PE-matmul-based weighted segment sums
(host-built one-hot selector matrices), then gathers the full output.
"""

import math
from contextlib import ExitStack
from dataclasses import dataclass

import ml_dtypes
import numpy as np

import concourse.bass as bass
import concourse.mybir as mybir
import concourse.tile as tile
from concourse import bacc
from concourse.bass_utils import run_bass_kernel_spmd
from concourse.masks import make_identity

BF16 = ml_dtypes.bfloat16
FP8 = ml_dtypes.float8_e4m3fn
AF = mybir.ActivationFunctionType


@dataclass(frozen=True)
class Cfg:
    n_nodes: int = 50000
    n_edges: int = 800000
    in_dim: int = 512
    h1: int = 512
    h2: int = 256
    out_dim: int = 128
    n_cores: int = 8
    group_blocks: int = 3  # row-blocks per gather group

    @property
    def nodes_per_core(self):
        return math.ceil(self.n_nodes / self.n_cores)

    @property
    def npad(self):  # per-core padded nodes
        return math.ceil(self.nodes_per_core / 128) * 128

    @property
    def nblocks(self):
        return self.npad // 128

    @property
    def ntot(self):
        return self.npad * self.n_cores

    @property
    def half(self):
        return self.ntot // 2

    @property
    def ngroups(self):
        return math.ceil(self.nblocks / self.group_blocks)


FULL = Cfg()


# ---------------------------------------------------------------- host prep


def edge_structure(cfg: Cfg, edge_row, edge_col, edge_weight):
    """Per-core edge streams with SPMD-uniform chunk counts.

    Returns (meta, per_core) where meta has the uniform chunk structure:
      meta['nch'][g][h]      total chunks in gather call (group g, half h)
      meta['chunk_blk'][g][h] list of block ids (one per chunk, ordered)
      meta['off16'][g][h]    idx-tile column offset (int16 cols) of the call
      meta['totch']          total chunks
      meta['idxcols']        total int16 columns of the idx tensor
    per_core[c] = dict(idx=[128, idxcols] int16, pmat=[128, totch*128] bf16)
    """
    nc_, npad, half, nb, G = (
        cfg.n_cores,
        cfg.npad,
        cfg.half,
        cfg.nblocks,
        cfg.group_blocks,
    )
    npc = cfg.nodes_per_core
    assert half <= 32767, "half-table must fit int16 indices"

    core_of = edge_row // npc
    lr_all = edge_row - core_of * npc  # local row
    # Block-aligned sub-table split: half 0 (A) holds every core's first
    # ceil(nb/2) row-blocks, half 1 (B) the rest, so table-half A can be
    # AllGathered as soon as each core finishes its first 25 blocks.
    # cl = position within the half-table.
    hh = ((nb + 1) // 2) * 128
    hB = npad - hh
    assert nc_ * hh <= 32767
    core_col = edge_col // npc
    loc = edge_col - core_col * npc
    half_all = (loc >= hh).astype(np.int64)
    cl_all = np.where(half_all == 0, core_col * hh + loc,
                      core_col * hB + (loc - hh))

    # bucket edges per (core, block, half)
    per = {}
    counts = np.zeros((nc_, nb, 2), np.int64)
    for c in range(nc_):
        m = core_of == c
        lr, cl, hf, w = lr_all[m], cl_all[m], half_all[m], edge_weight[m]
        blk = lr // 128
        order = np.lexsort((lr, hf, blk))
        per[c] = (lr[order], cl[order], hf[order], w[order], blk[order])
        np.add.at(counts[c], (blk, hf), 1)

    # uniform chunks per (block, half) = max over cores
    chunks_bh = np.ceil(counts / 128.0).astype(np.int64).max(axis=0)  # [nb, 2]

    ngroups = cfg.ngroups
    nch = [[0, 0] for _ in range(ngroups)]
    chunk_blk = [[[], []] for _ in range(ngroups)]
    off16 = [[0, 0] for _ in range(ngroups)]
    tot16 = 0
    totch = 0
    for g in range(ngroups):
        blocks = range(g * G, min((g + 1) * G, nb))
        for h in (0, 1):
            off16[g][h] = tot16
            n = 0
            for b in blocks:
                cb = int(chunks_bh[b, h])
                chunk_blk[g][h].extend([b] * cb)
                n += cb
            nch[g][h] = n
            tot16 += n * 8  # 128 idx per chunk -> 8 int16 cols
            totch += n

    meta = dict(
        nch=nch,
        chunk_blk=chunk_blk,
        off16=off16,
        totch=totch,
        idxcols=max(tot16, 8),
        chunks_bh=chunks_bh,
    )

    # SPMD-uniform chunk order within each (group, half): ascending
    # chunk-index-within-block (j), block-major within each j. Low-j chunks
    # are full for every core; high-j chunks carry the pad for every core,
    # so the pad concentrates at the call tail uniformly.
    # chunk_blk is rebuilt to match this order.
    for g in range(ngroups):
        blocks = list(range(g * G, min((g + 1) * G, nb)))
        for h in (0, 1):
            order = []
            maxcb = max((int(chunks_bh[b, h]) for b in blocks), default=0)
            for j in range(maxcb):
                for b in blocks:
                    if j < int(chunks_bh[b, h]):
                        order.append((b, j))
            meta["chunk_blk"][g][h] = [b for b, _ in order]
            meta.setdefault("chunk_ord", {})[(g, h)] = order

    # Pad slots: idx=-1 only in the TRAILING run of each <=15-chunk
    # sub-call (the gather ucode trims trailing negatives before
    # descriptor emission — saves Q7 time and DMA). Mid-call pad and the
    # first SAFE_GROUPS groups (whose SBUF tiles are uninitialized and
    # could hold NaN bytes; later groups reuse tiles holding valid old
    # values) use idx=0: gathers row 0, killed by zero pmat weight.
    SAFE_GROUPS = 999  # -1 trim disabled: trimmed calls hang the DMA sem protocol
    per_core = []
    for c in range(nc_):
        lr, cl, hf, w, blk = per[c]
        idx_flat = np.zeros(meta["idxcols"] * 16, np.int16)
        pmat = np.zeros((128, totch * 128), FP8)
        jchunk = 0
        for g in range(ngroups):
            blocks = list(range(g * G, min((g + 1) * G, nb)))
            for h in (0, 1):
                bdata = {}
                for b in blocks:
                    sel = (blk == b) & (hf == h)
                    bdata[b] = (cl[sel], lr[sel], w[sel])
                order = meta["chunk_ord"][(g, h)]
                ncall = len(order)
                base16 = meta["off16"][g][h]
                vals = np.zeros(ncall * 128, np.int64)
                real = np.zeros(ncall * 128, bool)
                for jj, (b, j) in enumerate(order):
                    e_cl, e_lr, e_w = bdata[b]
                    lo, hi = j * 128, min(j * 128 + 128, len(e_cl))
                    nreal = max(hi - lo, 0)
                    if nreal > 0:
                        vals[jj * 128 : jj * 128 + nreal] = e_cl[lo:hi]
                        real[jj * 128 : jj * 128 + nreal] = True
                        slot = np.arange(nreal)
                        r = e_lr[lo:hi] - b * 128
                        pmat[slot, (jchunk + jj) * 128 + r] = e_w[lo:hi].astype(
                            FP8
                        )
                jchunk += ncall
                if g >= SAFE_GROUPS:
                    # trailing trim per sub-call segment of 15 chunks
                    for seg in range(0, ncall, 15):
                        a, bnd = seg * 128, min(seg + 15, ncall) * 128
                        rseg = real[a:bnd]
                        nz = np.nonzero(rseg)[0]
                        last = nz[-1] + 1 if len(nz) else 0
                        # keep >=128 idx so every SDMA engine gets
                        # descriptors and the +16 completion sem fires
                        last = max(last, min(128, bnd - a))
                        vals[a + last : bnd] = -1
                i_in = np.arange(ncall * 128)
                idx_flat[(base16 + i_in // 16) * 16 + (i_in % 16)] = vals.astype(
                    np.int16
                )
        idx_mat = idx_flat.reshape(meta["idxcols"], 16).T  # [16, idxcols]
        idx_mat = np.tile(idx_mat, (8, 1))  # replicate to 128 partitions
        per_core.append(dict(idx=np.ascontiguousarray(idx_mat), pmat=pmat))

    return meta, per_core


def prep_inputs(cfg: Cfg, inputs):
    """Shard + lay out all per-core input tensors."""
    f = inputs["features"].astype(np.float32)
    meta, per_edge = edge_structure(
        cfg,
        inputs["edge_row"].astype(np.int64),
        inputs["edge_col"].astype(np.int64),
        inputs["edge_weight"].astype(np.float32),
    )
    kin = cfg.in_dim // 128
    k1 = cfg.h1 // 128
    k2 = cfg.h2 // 128

    def wlayout(w, kt):
        # [K, M] -> [128, kt*M] with [:, i*M:(i+1)*M] = w[i*128:(i+1)*128]
        K, M = w.shape
        return (
            w.reshape(kt, 128, M).transpose(1, 0, 2).reshape(128, kt * M)
        ).astype(BF16)

    w1 = wlayout(inputs["W_lin1"].astype(np.float32), kin)
    wg1 = wlayout(inputs["W_g1"].astype(np.float32), k1)
    wg2 = wlayout(inputs["W_g2"].astype(np.float32), k2)
    wl2 = wlayout(inputs["W_lin2"].astype(np.float32), k2)
    b1 = (
        inputs["b_lin1"].astype(np.float32).reshape(kin, 128).T.copy()
    )  # [128, kin]
    bg1 = np.tile(inputs["b_g1"].astype(np.float32).reshape(1, cfg.h2), (128, 1))
    bg2 = np.tile(inputs["b_g2"].astype(np.float32).reshape(1, cfg.h2), (128, 1))
    bl2 = np.tile(inputs["b_lin2"].astype(np.float32).reshape(1, cfg.out_dim), (128, 1))

    npc, npad = cfg.nodes_per_core, cfg.npad
    in_maps = []
    for c in range(cfg.n_cores):
        lo = c * npc
        hi = min((c + 1) * npc, cfg.n_nodes)
        xc = np.zeros((npad, cfg.in_dim), np.float32)
        xc[: hi - lo] = f[lo:hi]
        # XT layout [128, kin*npad]: [:, i*npad:(i+1)*npad] = x.T[i*128:...]
        xt = (
            xc.T.reshape(kin, 128, npad)
            .transpose(1, 0, 2)
            .reshape(128, kin * npad)
        ).astype(BF16)
        in_maps.append(
            {
                "xt": np.ascontiguousarray(xt),
                "w1": w1,
                "wg1": wg1,
                "wg2": wg2,
                "wl2": wl2,
                "b1": b1,
                "bg1": bg1,
                "bg2": bg2,
                "bl2": bl2,
                "idx": per_edge[c]["idx"],
                "pmat": per_edge[c]["pmat"],
            }
        )
    return meta, in_maps


# ---------------------------------------------------------------- kernel IR


def build(cfg: Cfg, meta):
    nc = bacc.Bacc(
        "TRN2",
        target_bir_lowering=False,
        debug=False,
        num_devices=cfg.n_cores,
        num_swdge_queues=4,
    )
    bf = mybir.dt.bfloat16
    f8 = mybir.dt.float8e4
    f32 = mybir.dt.float32
    i16 = mybir.dt.int16
    kin = cfg.in_dim // 128
    k1 = cfg.h1 // 128
    k2 = cfg.h2 // 128
    npad, nb, G, H2, OUT = (
        cfg.npad,
        cfg.nblocks,
        cfg.group_blocks,
        cfg.h2,
        cfg.out_dim,
    )
    HALF = cfg.half
    totch = meta["totch"]

    xt_d = nc.dram_tensor("xt", [128, kin * npad], bf, kind="ExternalInput").ap()
    w1_d = nc.dram_tensor("w1", [128, kin * cfg.h1], bf, kind="ExternalInput").ap()
    wg1_d = nc.dram_tensor("wg1", [128, k1 * H2], bf, kind="ExternalInput").ap()
    wg2_d = nc.dram_tensor("wg2", [128, k2 * H2], bf, kind="ExternalInput").ap()
    wl2_d = nc.dram_tensor("wl2", [128, k2 * OUT], bf, kind="ExternalInput").ap()
    b1_d = nc.dram_tensor("b1", [128, kin], f32, kind="ExternalInput").ap()
    bg1_d = nc.dram_tensor("bg1", [128, H2], f32, kind="ExternalInput").ap()
    bg2_d = nc.dram_tensor("bg2", [128, H2], f32, kind="ExternalInput").ap()
    bl2_d = nc.dram_tensor("bl2", [128, OUT], f32, kind="ExternalInput").ap()
    idx_d = nc.dram_tensor(
        "idx", [128, meta["idxcols"]], i16, kind="ExternalInput"
    ).ap()
    pmat_d = nc.dram_tensor(
        "pmat", [128, totch * 128], f8, kind="ExternalInput"
    ).ap()
    y_d = nc.dram_tensor("y", [npad, OUT], f32, kind="ExternalOutput").ap()

    hh = ((nb + 1) // 2) * 128  # A-half rows per core (block aligned)
    hB = npad - hh
    nbA = hh // 128  # blocks in A half
    HA = cfg.n_cores * hh
    HB = cfg.n_cores * hB

    g1_localA = nc.dram_tensor("g1_localA", [hh, H2], f8).ap()
    g1_localB = nc.dram_tensor("g1_localB", [hB, H2], f8).ap()
    g2_localA = nc.dram_tensor("g2_localA", [hh, H2], f8).ap()
    g2_localB = nc.dram_tensor("g2_localB", [hB, H2], f8).ap()
    g1_tableA = nc.dram_tensor("g1_tableA", [HA, H2], f8, addr_space="Shared").ap()
    g1_tableB = nc.dram_tensor("g1_tableB", [HB, H2], f8, addr_space="Shared").ap()
    g2_tableA = nc.dram_tensor("g2_tableA", [HA, H2], f8, addr_space="Shared").ap()
    g2_tableB = nc.dram_tensor("g2_tableB", [HB, H2], f8, addr_space="Shared").ap()

    rg = [list(range(cfg.n_cores))]

    def spmm(tc, ctx, nc, tables, idx_s, ones_t, brow, psum_tag, out_cb,
             after_group=None):
        """Weighted segment-sum of gathered table rows, per row-block.

        h0 gathers are issued S groups ahead of h1 so the first h1 gather
        (which waits for the B-half AllGather) doesn't starve the pipeline.
        """
        S = 6
        gp = [
            ctx.enter_context(
                tc.tile_pool(name=f"gath{psum_tag}{h}", bufs=(S + 2 if h == 0 else 4))
            )
            for h in (0, 1)
        ]
        pp = ctx.enter_context(tc.tile_pool(name=f"pm{psum_tag}", bufs=4))
        sp = ctx.enter_context(
            tc.tile_pool(name=f"ps{psum_tag}", bufs=2 * G, space="PSUM")
        )
        qstate = [0]

        def issue(g, h):
            n = meta["nch"][g][h]
            if n == 0:
                return None
            t = gp[h].tile([128, n, H2], f8, tag=f"g{h}")
            # split into <=15-chunk (1920-idx) sub-calls: a single
            # dma_gather must fit the SWDGE descriptor ring. Rotate
            # across the 4 SWDGE queues so descriptor generation runs
            # on all 4 Q7 core pairs concurrently.
            for lo in range(0, n, 15):
                ns = min(15, n - lo)
                o16 = meta["off16"][g][h] + lo * 8
                nc.gpsimd.dma_gather(
                    out_ap=t[:, lo : lo + ns, :],
                    in_ap=tables[h][:, :],
                    idxs_ap=idx_s[:, o16 : o16 + ns * 8],
                    num_idxs=ns * 128,
                    num_idxs_reg=ns * 128,
                    elem_size=H2,
                    single_packet=False,
                    queue_num=qstate[0] % 4,
                )
                qstate[0] += 1
            return t

        pend0 = {}
        for g in range(min(S, cfg.ngroups)):
            pend0[g] = issue(g, 0)
        j0 = 0
        for g in range(cfg.ngroups):
            blocks = list(range(g * G, min((g + 1) * G, nb)))
            gt = {0: pend0.pop(g)}
            if g + S < cfg.ngroups:
                pend0[g + S] = issue(g + S, 0)
            gt[1] = issue(g, 1)
            chg = meta["nch"][g][0] + meta["nch"][g][1]
            if chg > 0:
                ptile = pp.tile([128, chg * 128], f8, tag="p")
                nc.sync.dma_start(
                    ptile[:], pmat_d[:, j0 * 128 : (j0 + chg) * 128]
                )
            psums = {
                b: sp.tile([128, H2], f32, tag="ps", name=f"ps{psum_tag}_{b}")
                for b in blocks
            }
            last_jj = {}
            jj = 0
            for h in (0, 1):
                for b in meta["chunk_blk"][g][h]:
                    last_jj[b] = jj
                    jj += 1
            started = dict.fromkeys(blocks, False)
            jj = 0
            for h in (0, 1):
                for jh, b in enumerate(meta["chunk_blk"][g][h]):
                    nc.tensor.matmul(
                        psums[b][:],
                        lhsT=ptile[:, jj * 128 : (jj + 1) * 128],
                        rhs=gt[h][:, jh, :],
                        start=not started[b],
                        stop=(jj == last_jj[b]),
                    )
                    started[b] = True
                    jj += 1
            for b in blocks:
                assert started[b], f"block {b} has no chunks"
                out_cb(b, psums[b])
            if after_group is not None:
                after_group(g, blocks)
            j0 += chg

    with tile.TileContext(nc) as tc:
        with ExitStack() as top:
            const = top.enter_context(tc.tile_pool(name="const", bufs=1))
            w1_s = const.tile([128, kin * cfg.h1], bf)
            nc.sync.dma_start(w1_s[:], w1_d[:, :])
            wg1_s = const.tile([128, k1 * H2], bf)
            nc.sync.dma_start(wg1_s[:], wg1_d[:, :])
            wg2_s = const.tile([128, k2 * H2], bf)
            nc.sync.dma_start(wg2_s[:], wg2_d[:, :])
            wl2_s = const.tile([128, k2 * OUT], bf)
            nc.sync.dma_start(wl2_s[:], wl2_d[:, :])
            b1_s = const.tile([128, kin], f32)
            nc.sync.dma_start(b1_s[:], b1_d[:, :])
            bg1_s = const.tile([128, H2], f32)
            nc.sync.dma_start(bg1_s[:], bg1_d[:, :])
            bg2_s = const.tile([128, H2], f32)
            nc.sync.dma_start(bg2_s[:], bg2_d[:, :])
            bl2_s = const.tile([128, OUT], f32)
            nc.sync.dma_start(bl2_s[:], bl2_d[:, :])
            idx_s = const.tile([128, meta["idxcols"]], i16)
            nc.sync.dma_start(idx_s[:], idx_d[:, :])
            ident = const.tile([128, 128], bf)
            make_identity(nc, ident[:])
            ones_t = const.tile([1, 128], bf)
            nc.gpsimd.memset(ones_t[:], 1.0)

            # ---------------- L1: h1T[f, n] = sigmoid(W1.T @ X.T + b1)
            with ExitStack() as ph1:
                h1p = ph1.enter_context(tc.tile_pool(name="h1t", bufs=1))
                h1t = h1p.tile([128, k1 * npad], bf)
                with ExitStack() as px:
                    xp = px.enter_context(tc.tile_pool(name="xt", bufs=1))
                    psp = px.enter_context(
                        tc.tile_pool(name="ps1", bufs=4, space="PSUM")
                    )
                    xt_k = []
                    for kt in range(kin):
                        xk = xp.tile([128, npad], bf, name=f"xt{kt}")
                        nc.sync.dma_start(
                            xk[:], xt_d[:, kt * npad : (kt + 1) * npad]
                        )
                        xt_k.append(xk)
                    nsl = [(i * 512, min((i + 1) * 512, npad)) for i in range(math.ceil(npad / 512))]
                    for f1t in range(k1):
                        for a, b_ in nsl:
                            nw = b_ - a
                            ps = psp.tile([128, 512], f32, tag="ps")
                            for kt in range(kin):
                                nc.tensor.matmul(
                                    ps[:, :nw],
                                    lhsT=w1_s[
                                        :,
                                        kt * cfg.h1
                                        + f1t * 128 : kt * cfg.h1
                                        + f1t * 128
                                        + 128,
                                    ],
                                    rhs=xt_k[kt][:, a:b_],
                                    start=(kt == 0),
                                    stop=(kt == kin - 1),
                                )
                            nc.scalar.activation(
                                h1t[:, f1t * npad + a : f1t * npad + b_],
                                ps[:, :nw],
                                AF.Sigmoid,
                                bias=b1_s[:, f1t : f1t + 1],
                            )

                # ---------------- L2a: g1[n, h2] = h1 @ Wg1  (lhsT = h1T)
                def store_half(local_a, local_b, b, tile_):
                    if b < nbA:
                        nc.sync.dma_start(
                            local_a[b * 128 : (b + 1) * 128, :], tile_[:]
                        )
                    else:
                        bb = b - nbA
                        nc.sync.dma_start(
                            local_b[bb * 128 : (bb + 1) * 128, :], tile_[:]
                        )

                def allgather(ins_, outs_):
                    nc.gpsimd.collective_compute(
                        "AllGather",
                        mybir.AluOpType.bypass,
                        replica_groups=rg,
                        ins=[ins_],
                        outs=[outs_],
                    )

                with ExitStack() as p2:
                    psp2 = p2.enter_context(
                        tc.tile_pool(name="ps2", bufs=4, space="PSUM")
                    )
                    tp2 = p2.enter_context(tc.tile_pool(name="g1t", bufs=3))
                    for b in range(nb):
                        ps = psp2.tile([128, H2], f32, tag="ps")
                        for kt in range(k1):
                            nc.tensor.matmul(
                                ps[:],
                                lhsT=h1t[
                                    :, kt * npad + b * 128 : kt * npad + b * 128 + 128
                                ],
                                rhs=wg1_s[:, kt * H2 : (kt + 1) * H2],
                                start=(kt == 0),
                                stop=(kt == k1 - 1),
                            )
                        g1tile = tp2.tile([128, H2], f8, tag="g1")
                        nc.vector.tensor_copy(g1tile[:], ps[:])
                        store_half(g1_localA, g1_localB, b, g1tile)
                        if b == nbA - 1:
                            allgather(g1_localA[:, :], g1_tableA[:, :])
                    allgather(g1_localB[:, :], g1_tableB[:, :])

            # ---------------- spmm1 -> h2, L3a (g2) fused per block,
            # AG2 halves issued as soon as their blocks are stored
            with ExitStack() as ph2:
                h2p = ph2.enter_context(tc.tile_pool(name="h2res", bufs=1))
                h2r = h2p.tile([128, nb * H2], bf)
                tps = ph2.enter_context(
                    tc.tile_pool(name="tps", bufs=1, space="PSUM")
                )
                psp3 = ph2.enter_context(
                    tc.tile_pool(name="ps3", bufs=1, space="PSUM")
                )
                tp3 = ph2.enter_context(tc.tile_pool(name="l3t", bufs=4))

                with ExitStack() as ps1:
                    def cb1(b, psum):
                        tmp = tp3.tile([128, H2], f32, tag="tmp")
                        nc.vector.tensor_add(tmp[:], psum[:], bg1_s[:])
                        nc.vector.tensor_scalar_max(
                            h2r[:, b * H2 : (b + 1) * H2], tmp[:], 0.0
                        )
                        h2T = tp3.tile([128, k2, 128], bf, tag="h2T")
                        for kt in range(k2):
                            pt = tps.tile([128, 128], bf, tag="pt")
                            nc.tensor.transpose(
                                pt[:],
                                h2r[:, b * H2 + kt * 128 : b * H2 + (kt + 1) * 128],
                                ident[:],
                            )
                            nc.vector.tensor_copy(h2T[:, kt, :], pt[:])
                        ps = psp3.tile([128, H2], f32, tag="ps")
                        for kt in range(k2):
                            nc.tensor.matmul(
                                ps[:],
                                lhsT=h2T[:, kt, :],
                                rhs=wg2_s[:, kt * H2 : (kt + 1) * H2],
                                start=(kt == 0),
                                stop=(kt == k2 - 1),
                            )
                        g2tile = tp3.tile([128, H2], f8, tag="g2")
                        nc.vector.tensor_copy(g2tile[:], ps[:])
                        store_half(g2_localA, g2_localB, b, g2tile)

                    spmm(tc, ps1, nc, (g1_tableA, g1_tableB), idx_s, ones_t,
                         bg1_s, "a", cb1)

                # both AG2 halves after the full spmm1/L3a stream: a
                # mid-stream collective trigger blocks the gpsimd queue on
                # cross-core rendezvous and stalls the gather pipeline
                allgather(g2_localA[:, :], g2_tableA[:, :])
                allgather(g2_localB[:, :], g2_tableB[:, :])

            # ---------------- spmm2 + L4 fused per block
            with ExitStack() as ps2x:
                tps4 = ps2x.enter_context(
                    tc.tile_pool(name="tps4", bufs=1, space="PSUM")
                )
                psp4 = ps2x.enter_context(
                    tc.tile_pool(name="ps4", bufs=1, space="PSUM")
                )
                tp4 = ps2x.enter_context(tc.tile_pool(name="l4t", bufs=3))

                def cb2(b, psum):
                    tmp = tp4.tile([128, H2], f32, tag="tmp4")
                    nc.vector.tensor_add(tmp[:], psum[:], bg2_s[:])
                    h3t = tp4.tile([128, H2], bf, tag="h3")
                    nc.vector.tensor_scalar_max(h3t[:], tmp[:], 0.0)
                    h3T = tp4.tile([128, k2, 128], bf, tag="h3T")
                    for kt in range(k2):
                        pt = tps4.tile([128, 128], bf, tag="pt")
                        nc.tensor.transpose(
                            pt[:], h3t[:, kt * 128 : (kt + 1) * 128], ident[:]
                        )
                        nc.vector.tensor_copy(h3T[:, kt, :], pt[:])
                    ps4 = psp4.tile([128, OUT], f32, tag="ps")
                    for kt in range(k2):
                        nc.tensor.matmul(
                            ps4[:],
                            lhsT=h3T[:, kt, :],
                            rhs=wl2_s[:, kt * OUT : (kt + 1) * OUT],
                            start=(kt == 0),
                            stop=(kt == k2 - 1),
                        )
                    yt = tp4.tile([128, OUT], f32, tag="y")
                    nc.vector.tensor_add(yt[:], ps4[:], bl2_s[:])
                    nc.sync.dma_start(y_d[b * 128 : (b + 1) * 128, :], yt[:])

                spmm(tc, ps2x, nc, (g2_tableA, g2_tableB), idx_s, ones_t,
                     bg2_s, "b", cb2)

    nc.compile()
    return nc


# ---------------------------------------------------------------- driver

_CACHE = {}


def run(inputs, cfg: Cfg = FULL, trace=False, tmpdir=None):
    meta, in_maps = prep_inputs(cfg, inputs)
    key = (cfg, meta["totch"], meta["idxcols"])
    if key not in _CACHE:
        _CACHE[key] = build(cfg, meta)
    nc = _CACHE[key]
    res = run_bass_kernel_spmd(
        nc,
        in_maps,
        core_ids=list(range(cfg.n_cores)),
        trace=trace,
        tmpdir=tmpdir,
    )
    npc = cfg.nodes_per_core
    out = np.empty((cfg.n_nodes, cfg.out_dim), np.float32)
    for c in range(cfg.n_cores):
        lo = c * npc
        hi = min((c + 1) * npc, cfg.n_nodes)
        out[lo:hi] = res.results[c]["y"][: hi - lo]
    return out, res


def kernel(**inputs) -> np.ndarray:
    out, _ = run(inputs, FULL, trace=False)
    return out

